# revision 1
# baseline (speedup 1.0000x reference)
"""DepletionLSTM Trainium2 kernel.

Self-contained: builds a Bass/Tile kernel for the 2-layer-LSTM network,
shards the batch over 8 NeuronCores (pure data parallelism), runs via
PJRT/axon, returns the full [8192, 30] float32 output.

Strategy (per core, 1024 batch):
- Everything resident in SBUF; no DRAM round-trips for activations.
- Feature-major layout: activations are [H=128 partitions, batch] tiles.
- Input-projection LayerNorm stats are computed in a prepass directly in
  [T=90 partitions, batch] layout using the quadratic-form identity
  sum_h p_h^2 = x^T (W^T W) x + 2 (W^T b)^T x + |b|^2 (F=7 is tiny, so the
  F-contractions are unrolled on the vector engine).  rsqrt is batched into
  a single Sqrt activation so the ACT table never switches inside the loop.
- Per step: x_t is PE-transposed to feature-major and pre-scaled by rstd
  (LN scaling commutes through the projection matmul); the projection plus a
  K=2 rank-2 term (b_in*rstd and -mean*rstd rows) accumulates in PSUM and a
  single DVE copy produces the normalized LSTM input.  Each LSTM layer is 4
  accumulating gate matmul pairs (input + recurrent), 4 sigmoid/tanh ACT ops
  with the gate bias folded into the activation bias, tanh(c), and 4 DVE
  elementwise ops.  Layer 1 runs one timestep behind layer 0 (double-buffered
  h0) so both layers' engine work overlaps.
- Matmul operands use float32r (fp32 bytes, single-pass PE) for speed.

PSUM (8 banks): "pg" gates/head 2x[128,1024] (4), "pp" projection [128,1024]
(2), "pxt" x-transposes 2x[7,512] (2).
"""
import sys
sys.path.insert(0, '/opt/trn_rl_repo')

import numpy as np

B, T, F, H, D1, D2, OUT = 8192, 90, 7, 128, 128, 64, 30
NCORES = 8
BL = B // NCORES
G4 = 4 * H
NH = BL // 512
QB = BL // 128
EPS = 1e-5
MMDT = "float32r"
V_ON_POOL = False
XFMR_ON_POOL = False
PGBUFS = 2


def _build(nc, T_steps=T, mmdt_name=MMDT, dbg=False):
    global V_ON_POOL, XFMR_ON_POOL, PGBUFS
    import concourse.tile as tile
    from concourse import mybir
    from concourse.masks import make_identity

    f32 = mybir.dt.float32
    mmdt = getattr(mybir.dt, mmdt_name)
    AF = mybir.ActivationFunctionType
    ALU = mybir.AluOpType

    # ---------------- DRAM I/O ----------------
    x_d = nc.dram_tensor("x", [BL, T, F], f32, kind="ExternalInput")
    W_in_d = nc.dram_tensor("W_in", [H, F], f32, kind="ExternalInput")
    b_in_d = nc.dram_tensor("b_in", [H], f32, kind="ExternalInput")
    g_in_d = nc.dram_tensor("g_in", [H], f32, kind="ExternalInput")
    be_in_d = nc.dram_tensor("be_in", [H], f32, kind="ExternalInput")
    Wih_d = [nc.dram_tensor("Wih0", [G4, H], f32, kind="ExternalInput"),
             nc.dram_tensor("Wih1", [G4, H], f32, kind="ExternalInput")]
    Whh_d = [nc.dram_tensor("Whh0", [G4, H], f32, kind="ExternalInput"),
             nc.dram_tensor("Whh1", [G4, H], f32, kind="ExternalInput")]
    bih_d = [nc.dram_tensor("bih0", [G4], f32, kind="ExternalInput"),
             nc.dram_tensor("bih1", [G4], f32, kind="ExternalInput")]
    bhh_d = [nc.dram_tensor("bhh0", [G4], f32, kind="ExternalInput"),
             nc.dram_tensor("bhh1", [G4], f32, kind="ExternalInput")]
    g_ln_d = nc.dram_tensor("g_ln", [H], f32, kind="ExternalInput")
    be_ln_d = nc.dram_tensor("be_ln", [H], f32, kind="ExternalInput")
    W_d1_d = nc.dram_tensor("W_d1", [D1, H], f32, kind="ExternalInput")
    b_d1_d = nc.dram_tensor("b_d1", [D1], f32, kind="ExternalInput")
    W_d2_d = nc.dram_tensor("W_d2", [D2, D1], f32, kind="ExternalInput")
    b_d2_d = nc.dram_tensor("b_d2", [D2], f32, kind="ExternalInput")
    W_d3_d = nc.dram_tensor("W_d3", [OUT, D2], f32, kind="ExternalInput")
    b_d3_d = nc.dram_tensor("b_d3", [OUT], f32, kind="ExternalInput")
    out_d = nc.dram_tensor("out", [BL, OUT], f32, kind="ExternalOutput")
    if dbg:
        dbg_xfm = nc.dram_tensor("dbg_xfm", [F, BL], f32, kind="ExternalOutput")
        dbg_stats = nc.dram_tensor("dbg_stats", [2, BL], f32, kind="ExternalOutput")
        dbg_x0 = nc.dram_tensor("dbg_x0", [H, BL], f32, kind="ExternalOutput")
        dbg_h0 = nc.dram_tensor("dbg_h0", [H, BL], f32, kind="ExternalOutput")
        dbg_c0 = nc.dram_tensor("dbg_c0", [H, BL], f32, kind="ExternalOutput")
        dbg_pp = nc.dram_tensor("dbg_pp", [H, BL], f32, kind="ExternalOutput")
        dbg_rbc = nc.dram_tensor("dbg_rbc", [2, BL], f32, kind="ExternalOutput")

    import contextlib
    with tile.TileContext(nc) as tc, contextlib.ExitStack() as ctx:
        singles = ctx.enter_context(tc.tile_pool(name="singles", bufs=1))
        trans = ctx.enter_context(tc.tile_pool(name="trans", bufs=2))
        small = ctx.enter_context(tc.tile_pool(name="small", bufs=2))
        ps_pg = ctx.enter_context(tc.tile_pool(name="ps_pg", bufs=PGBUFS, space="PSUM"))
        ps_pp = ctx.enter_context(tc.tile_pool(name="ps_pp", bufs=1, space="PSUM"))
        ps_px = ctx.enter_context(tc.tile_pool(name="ps_px", bufs=2, space="PSUM"))
        dpool = ctx.enter_context(tc.tile_pool(name="dpool", bufs=1, space="DRAM"))

        def pg_tile(shape, name):
            return ps_pg.tile(shape, f32, tag="pg", name=name)

        def pp_tile(shape, name):
            return ps_pp.tile(shape, f32, tag="pp", name=name)

        def px_tile(shape, name):
            return ps_px.tile(shape, f32, tag="pxt", name=name)

        def R(ap):
            return ap

        # ---------------- constants ----------------
        ident = singles.tile([128, 128], f32)
        make_identity(nc, ident)
        ones_row = singles.tile([1, 512], f32)
        nc.vector.memset(ones_row, 1.0)
        ones_col = singles.tile([128, 1], f32)
        nc.vector.memset(ones_col, 1.0)
        eps_col = singles.tile([T, 1], f32)
        nc.vector.memset(eps_col, EPS)

        def load_col(dram_vec, n, name):
            t_ = singles.tile([n, 1], f32, name=name, tag=name)
            nc.sync.dma_start(out=t_, in_=dram_vec[:].rearrange("(p o) -> p o", o=1))
            return t_

        g_in_c = load_col(g_in_d, H, "g_in_c")
        be_in_c = load_col(be_in_d, H, "be_in_c")
        b_in_c = load_col(b_in_d, H, "b_in_c")
        g_ln_c = load_col(g_ln_d, H, "g_ln_c")
        be_ln_c = load_col(be_ln_d, H, "be_ln_c")
        b_d1_c = load_col(b_d1_d, D1, "b_d1_c")
        b_d2_c = load_col(b_d2_d, D2, "b_d2_c")
        b_d3_c = load_col(b_d3_d, OUT, "b_d3_c")
        b_in_row = singles.tile([1, H], f32)
        nc.sync.dma_start(out=b_in_row, in_=b_in_d[:].rearrange("(o p) -> o p", o=1))
        bn1_dram = dpool.tile([2, H], f32)
        nc.sync.dma_start(out=bn1_dram[0:1, :],
                          in_=b_in_d[:].rearrange("(o p) -> o p", o=1))
        nc.sync.dma_start(out=bn1_dram[1:2, :], in_=ones_row[:, 0:H])
        bn1 = singles.tile([2, H], f32)
        nc.sync.dma_start(out=bn1, in_=bn1_dram[:, :])

        # ---------------- weights: load + PE-transpose ----------------
        def transpose_to(dst, src_ap, p, fdim):
            pt = pp_tile([fdim, p], "tr_ps")
            nc.tensor.transpose(pt, src_ap, ident[:p, :p])
            nc.vector.tensor_copy(out=dst, in_=pt)

        w_in_raw = singles.tile([H, F], f32)
        nc.sync.dma_start(out=w_in_raw, in_=W_in_d[:, :])
        w_inT = singles.tile([F, H], mmdt)
        transpose_to(w_inT, w_in_raw, H, F)

        wihT0f = singles.tile([H, 4, H], f32)
        wihT, whhT = [], []
        for L in range(2):
            wt = singles.tile([H, 4, H], mmdt, name=f"wihT{L}", tag=f"wihT{L}")
            ht = singles.tile([H, 4, H], mmdt, name=f"whhT{L}", tag=f"whhT{L}")
            for cc in range(4):
                raw = trans.tile([H, H], f32, tag="u", name="raw")
                nc.sync.dma_start(out=raw, in_=Wih_d[L][cc * H:(cc + 1) * H, :])
                pt_w = pp_tile([H, H], "tr_ps_w")
                nc.tensor.transpose(pt_w, raw, ident)
                nc.vector.tensor_copy(out=wt[:, cc, :], in_=pt_w)
                if L == 0:
                    nc.vector.tensor_copy(out=wihT0f[:, cc, :], in_=pt_w)
                raw2 = trans.tile([H, H], f32, tag="v_", name="raw2")
                nc.sync.dma_start(out=raw2, in_=Whh_d[L][cc * H:(cc + 1) * H, :])
                transpose_to(ht[:, cc, :], raw2, H, H)
            wihT.append(wt)
            whhT.append(ht)

        # gate biases beff[L] [128, 4]; layer-0 gains Wih0 @ be_in (beta fold)
        beff = []
        for L in range(2):
            bt_ = singles.tile([H, 4], f32, name=f"beff{L}", tag=f"beff{L}")
            bih_sb = small.tile([H, 4], f32, tag="bload", name="bih_sb")
            nc.sync.dma_start(out=bih_sb,
                              in_=bih_d[L][:].rearrange("(c p) -> p c", p=H))
            bhh_sb = small.tile([H, 4], f32, tag="bload2", name="bhh_sb")
            nc.sync.dma_start(out=bhh_sb,
                              in_=bhh_d[L][:].rearrange("(c p) -> p c", p=H))
            nc.vector.tensor_add(out=bt_, in0=bih_sb, in1=bhh_sb)
            beff.append(bt_)
        for cc in range(4):
            pb = px_tile([H, 1], "pb")
            nc.tensor.matmul(pb, wihT0f[:, cc, :], be_in_c, start=True, stop=True)
            nc.vector.tensor_add(out=beff[0][:, cc:cc + 1],
                                 in0=beff[0][:, cc:cc + 1], in1=pb)
        # gamma-fold layer-0 input weights (rows scaled by g_in)
        nc.vector.tensor_scalar_mul(
            out=wihT[0][:, :, :].rearrange("p c m -> p (c m)"),
            in0=wihT[0][:, :, :].rearrange("p c m -> p (c m)"),
            scalar1=g_in_c)

        wd1T = singles.tile([H, D1], f32)
        wd1_raw = trans.tile([D1, H], f32, tag="u", name="wd1_raw")
        nc.sync.dma_start(out=wd1_raw, in_=W_d1_d[:, :])
        transpose_to(wd1T, wd1_raw, D1, H)
        wd2T = singles.tile([D1, D2], f32)
        wd2_raw = trans.tile([D2, D1], f32, tag="v_", name="wd2_raw")
        nc.sync.dma_start(out=wd2_raw, in_=W_d2_d[:, :])
        transpose_to(wd2T, wd2_raw, D2, D1)
        wd3T = singles.tile([D2, OUT], f32)
        wd3_raw = trans.tile([OUT, D2], f32, tag="u", name="wd3_raw")
        nc.sync.dma_start(out=wd3_raw, in_=W_d3_d[:, :])
        transpose_to(wd3T, wd3_raw, OUT, D2)

        # ---------------- x loads ----------------
        # loop layout: xrow[p, t, q, f] = x[128q+p, t, f]
        xrow_all = singles.tile([128, T, QB, F], f32)
        nc.sync.dma_start(
            out=xrow_all,
            in_=x_d[:, :, :].rearrange("(q p) t f -> p t q f", p=128))
        # prepass layout: x_tm[t, q, p, f] = x[128q+p, t, f]
        x_tm = singles.tile([T, QB, 128, F], f32)
        nc.sync.dma_start(
            out=x_tm,
            in_=x_d[:, :, :].rearrange("(q p) t f -> t q p f", p=128))

        # ---------------- prepass: LN stats in [T, BL] layout ----------------
        # p' = W_in x + b_in per (h | b,t); over h:
        #   sum p'   = wsum . x + bsum
        #   sum p'^2 = x^T M x + 2 l^T x + c0,  M = W^T W, l = W^T b, c0=|b|^2
        p_m = pp_tile([F, F], "stat_m")
        nc.tensor.matmul(p_m, w_in_raw, w_in_raw, start=True, stop=True)
        p_ws = px_tile([1, F], "stat_ws")
        nc.tensor.matmul(p_ws, ones_col, w_in_raw, start=True, stop=True)
        p_l = px_tile([1, F], "stat_l")
        nc.tensor.matmul(p_l, b_in_c, w_in_raw, start=True, stop=True)
        p_sc = px_tile([1, 2], "stat_sc")
        nc.tensor.matmul(p_sc[:, 0:1], b_in_c, b_in_c, start=True, stop=False,
                         skip_group_check=True)
        nc.tensor.matmul(p_sc[:, 1:2], ones_col, b_in_c, start=False, stop=True,
                         skip_group_check=True)
        m_sb = small.tile([F, F], f32, tag="m_sb", name="m_sb")
        nc.vector.tensor_copy(out=m_sb, in_=p_m)
        ws_sb = small.tile([1, F], f32, tag="ws_sb", name="ws_sb")
        nc.vector.tensor_copy(out=ws_sb, in_=p_ws)
        l_sb = small.tile([1, F], f32, tag="l_sb", name="l_sb")
        nc.vector.tensor_copy(out=l_sb, in_=p_l)
        sc_sb = small.tile([1, 2], f32, tag="sc_sb", name="sc_sb")
        nc.vector.tensor_copy(out=sc_sb, in_=p_sc)
        # stage stat constants to DRAM, then partition-broadcast them back
        stat_dram = dpool.tile([F + 2, F * F], f32)
        nc.sync.dma_start(out=stat_dram[0:1, :].rearrange("o (a b) -> (o a) b", a=F),
                          in_=m_sb)
        nc.sync.dma_start(out=stat_dram[F:F + 1, 0:F], in_=ws_sb)
        nc.sync.dma_start(out=stat_dram[F:F + 1, F:2 * F], in_=l_sb)
        nc.sync.dma_start(out=stat_dram[F + 1:F + 2, 0:2], in_=sc_sb)
        wbc = singles.tile([T, F], f32)
        nc.gpsimd.dma_start(out=wbc, in_=stat_dram[F:F + 1, 0:F].to_broadcast([T, F]))
        lbc = singles.tile([T, F], f32)
        nc.gpsimd.dma_start(out=lbc,
                            in_=stat_dram[F:F + 1, F:2 * F].to_broadcast([T, F]))
        mbc = singles.tile([T, F * F], f32)
        nc.gpsimd.dma_start(out=mbc, in_=stat_dram[0:1, :].to_broadcast([T, F * F]))
        scbc = singles.tile([T, 2], f32)
        nc.gpsimd.dma_start(out=scbc,
                            in_=stat_dram[F + 1:F + 2, 0:2].to_broadcast([T, 2]))

        def xf(fi):
            return x_tm[:T_steps, :, :, fi].rearrange("t q p -> t (q p)")

        TS = T_steps
        nmu_all = singles.tile([T, BL], f32)
        r_all = singles.tile([T, BL], f32)
        acc = trans.tile([T, BL], f32, tag="sig_i", name="st_acc")
        nc.vector.tensor_scalar_mul(out=acc[:TS], in0=xf(0), scalar1=wbc[:TS, 0:1])
        for fi in range(1, F):
            nc.vector.scalar_tensor_tensor(
                out=acc[:TS], in0=xf(fi), scalar=wbc[:TS, fi:fi + 1],
                in1=acc[:TS], op0=ALU.mult, op1=ALU.add)
        # nmu = -(acc + bsum)/H
        nc.vector.tensor_scalar(out=nmu_all[:TS], in0=acc[:TS],
                                scalar1=scbc[:TS, 1:2], scalar2=-1.0 / H,
                                op0=ALU.add, op1=ALU.mult)
        # quadratic form
        qacc = trans.tile([T, BL], f32, tag="sig_f", name="st_qacc")
        yf = trans.tile([T, BL], f32, tag="tg", name="st_yf")
        tmp = trans.tile([T, BL], f32, tag="sig_o", name="st_tmp")
        yf2 = trans.tile([T, BL], f32, tag="sig_o", name="st_yf2")
        qacc2 = trans.tile([T, BL], f32, tag="u", name="st_qacc2")
        tmp2 = trans.tile([T, BL], f32, tag="v_", name="st_tmp2")
        for fi in range(F):
            eng = nc.vector
            y_, q_, t_ = (yf, qacc, tmp) if eng is nc.vector else (yf2, qacc2, tmp2)
            eng.tensor_scalar_mul(out=y_[:TS], in0=xf(0),
                                  scalar1=mbc[:TS, fi * F:fi * F + 1])
            for fj in range(1, F):
                eng.scalar_tensor_tensor(
                    out=y_[:TS], in0=xf(fj),
                    scalar=mbc[:TS, fi * F + fj:fi * F + fj + 1],
                    in1=y_[:TS], op0=ALU.mult, op1=ALU.add)
            eng.tensor_tensor(out=t_[:TS], in0=xf(fi), in1=y_[:TS], op=ALU.mult)
            if fi == 0:
                nc.vector.tensor_copy(out=qacc[:TS], in_=t_[:TS])
            elif fi == 2:
                nc.vector.tensor_copy(out=qacc2[:TS], in_=t_[:TS])
            elif eng is nc.vector:
                nc.vector.tensor_add(out=qacc[:TS], in0=qacc[:TS], in1=t_[:TS])
            else:
                nc.vector.tensor_add(out=qacc2[:TS], in0=qacc2[:TS], in1=t_[:TS])
        nc.vector.tensor_add(out=qacc[:TS], in0=qacc[:TS], in1=qacc2[:TS])
        # + 2 l.x
        lin = trans.tile([T, BL], f32, tag="u", name="st_lin")
        nc.vector.tensor_scalar_mul(out=lin[:TS], in0=xf(0), scalar1=lbc[:TS, 0:1])
        for fi in range(1, F):
            nc.vector.scalar_tensor_tensor(
                out=lin[:TS], in0=xf(fi), scalar=lbc[:TS, fi:fi + 1],
                in1=lin[:TS], op0=ALU.mult, op1=ALU.add)
        nc.vector.scalar_tensor_tensor(out=qacc[:TS], in0=lin[:TS], scalar=2.0,
                                       in1=qacc[:TS], op0=ALU.mult, op1=ALU.add)
        # var = (q + c0)/H - mu^2 ; r = 1/sqrt(var+eps)
        nc.vector.tensor_scalar(out=qacc[:TS], in0=qacc[:TS],
                                scalar1=scbc[:TS, 0:1], scalar2=1.0 / H,
                                op0=ALU.add, op1=ALU.mult)
        nc.vector.tensor_tensor(out=tmp[:TS], in0=nmu_all[:TS], in1=nmu_all[:TS],
                                op=ALU.mult)
        nc.vector.tensor_sub(out=qacc[:TS], in0=qacc[:TS], in1=tmp[:TS])
        nc.scalar.activation(out=r_all[:TS], in_=qacc[:TS], func=AF.Sqrt,
                             bias=eps_col[:TS], scale=1.0)
        nc.vector.reciprocal(out=r_all[:TS], in_=r_all[:TS])
        nmr_all = singles.tile([T, BL], f32)
        nc.vector.tensor_tensor(out=nmr_all[:TS], in0=nmu_all[:TS],
                                in1=r_all[:TS], op=ALU.mult)
        rnm_dram = dpool.tile([2, T, BL], f32)
        nc.sync.dma_start(out=rnm_dram[0, :TS], in_=r_all[:TS])
        nc.sync.dma_start(out=rnm_dram[1, :TS], in_=nmr_all[:TS])
        r_dram = rnm_dram[0]

        # ---------------- states ----------------
        h1 = singles.tile([H, BL], mmdt, name="h1", tag="h1")
        c = [singles.tile([H, BL], f32, name="c0", tag="c0"),
             singles.tile([H, BL], f32, name="c1", tag="c1")]
        zinit = trans.tile([H, BL], f32, tag="x0", name="zinit")
        nc.vector.memset(zinit, 0.0)
        h0_prev = trans.tile([H, BL], mmdt, tag="h0", name="h0_init")
        nc.vector.tensor_copy(out=h0_prev, in_=zinit)
        nc.vector.tensor_copy(out=h1, in_=zinit)
        for L in range(2):
            nc.vector.memset(c[L], 0.0)

        # ---------------- main loop ----------------
        def lstm_step(L, inp, hprev, hout, hh_first):
            sig_i = trans.tile([H, BL], f32, tag="sig_i", name="sig_i")
            sig_f = trans.tile([H, BL], f32, tag="sig_f", name="sig_f")
            tg = trans.tile([H, BL], f32, tag="tg", name="tg")
            sig_o = trans.tile([H, BL], f32, tag="sig_o", name="sig_o")
            outs = [sig_i, sig_f, tg, sig_o]
            funcs = [AF.Sigmoid, AF.Sigmoid, AF.Tanh, AF.Sigmoid]
            for gc in range(4):
                pg = pg_tile([H, BL], "pg_gates")
                for hc in range(NH):
                    sl = slice(hc * 512, (hc + 1) * 512)
                    ops = [(wihT[L][:, gc, :], inp), (whhT[L][:, gc, :], hprev)]
                    if hh_first:
                        ops.reverse()
                    nc.tensor.matmul(pg[:, sl], R(ops[0][0]), R(ops[0][1][:, sl]),
                                     start=True, stop=False)
                    nc.tensor.matmul(pg[:, sl], R(ops[1][0]), R(ops[1][1][:, sl]),
                                     start=False, stop=True)
                nc.scalar.activation(out=outs[gc], in_=pg, func=funcs[gc],
                                     bias=beff[L][:, gc:gc + 1], scale=1.0)
            u = trans.tile([H, BL], f32, tag="u", name="u")
            nc.vector.tensor_tensor(out=u, in0=sig_i, in1=tg, op=ALU.mult)
            v_ = trans.tile([H, BL], f32, tag="v_", name="v_")
            (nc.gpsimd if V_ON_POOL else nc.vector).tensor_tensor(
                out=v_, in0=sig_f, in1=c[L], op=ALU.mult)
            nc.vector.tensor_add(out=c[L], in0=u, in1=v_)
            tc_ = trans.tile([H, BL], f32, tag="tc_", name="tc_")
            nc.scalar.activation(out=tc_, in_=c[L], func=AF.Tanh, scale=1.0)
            nc.vector.tensor_tensor(out=hout, in0=sig_o, in1=tc_, op=ALU.mult)

        for t in range(T_steps):
            # x_t -> feature-major [7, BL] via strided DMA (f-major gather)
            x_fm = trans.tile([F, BL], f32, tag="x_fm", name="x_fm")
            pxs = []
            for half in range(2):
                px = px_tile([F, 512], f"pxt{half}")
                for qi in range(4):
                    q = half * 4 + qi
                    nc.tensor.transpose(
                        px[:, qi * 128:(qi + 1) * 128],
                        xrow_all[:, t, q, :], ident)
                pxs.append(px)
            nc.vector.tensor_copy(out=x_fm[:, 0:512], in_=pxs[0])
            nc.vector.tensor_copy(out=x_fm[:, 512:1024], in_=pxs[1])
            # rstd rows: broadcast over 7 partitions + flat rows for rank-1s
            rbc7 = trans.tile([F, BL], f32, tag="rbc7", name="rbc7")
            nc.gpsimd.dma_start(out=rbc7,
                                in_=r_dram[t:t + 1, :].to_broadcast([F, BL]))
            rn = small.tile([2, BL], f32, tag="rn", name="rn")
            nc.gpsimd.dma_start(out=rn, in_=rnm_dram[:, t, :])
            # x_fm_r = x_fm * rstd (per column)
            x_fm_r = trans.tile([F, BL], mmdt, tag="x_fm_r", name="x_fm_r")
            (nc.gpsimd if XFMR_ON_POOL else nc.vector).tensor_tensor(
                out=x_fm_r, in0=x_fm, in1=rbc7, op=ALU.mult)
            # x0 = W_in @ x_fm_r + b_in x r_row + 1 x nmr_row  (PSUM)
            pp = pp_tile([H, BL], "pp_proj")
            for hc in range(NH):
                sl = slice(hc * 512, (hc + 1) * 512)
                nc.tensor.matmul(pp[:, sl], R(w_inT), R(x_fm_r[:, sl]),
                                 start=True, stop=False, skip_group_check=True)
                nc.tensor.matmul(pp[:, sl], bn1, rn[:, sl],
                                 start=False, stop=(hc == NH - 1),
                                 skip_group_check=True)
            x0 = trans.tile([H, BL], mmdt, tag="x0", name="x0")
            nc.vector.tensor_copy(out=x0, in_=pp)
            # layer 1 runs one step behind layer 0 (consumes h0 of step t-1)
            if t > 0:
                lstm_step(1, h0_prev, h1, h1, hh_first=True)
            h0_new = trans.tile([H, BL], mmdt, tag="h0", name="h0_new")
            lstm_step(0, x0, h0_prev, h0_new, hh_first=False)
            h0_prev = h0_new
            if dbg and t == 0:
                ppc = trans.tile([H, BL], f32, tag="tc_", name="ppc_dbg")
                nc.vector.tensor_copy(out=ppc, in_=pp)
                nc.sync.dma_start(out=dbg_pp[:, :], in_=ppc)
                nc.sync.dma_start(out=dbg_rbc[:, :], in_=rn)
                nc.sync.dma_start(out=dbg_xfm[:, :], in_=x_fm)
                nc.sync.dma_start(out=dbg_stats[0:1, :], in_=nmu_all[0:1, :])
                nc.sync.dma_start(out=dbg_stats[1:2, :], in_=r_all[0:1, :])
                nc.sync.dma_start(out=dbg_x0[:, :], in_=x0.bitcast(f32))
                nc.sync.dma_start(out=dbg_h0[:, :], in_=h0_new.bitcast(f32))
                nc.sync.dma_start(out=dbg_c0[:, :], in_=c[0])
        lstm_step(1, h0_prev, h1, h1, hh_first=True)

        # ---------------- head ----------------
        h1f = trans.tile([H, BL], f32, tag="x0", name="h1f")
        nc.vector.tensor_copy(out=h1f, in_=h1.bitcast(f32))
        sqh = trans.tile([H, BL], f32, tag="sig_f", name="sqh")
        nc.vector.tensor_tensor(out=sqh, in0=h1f, in1=h1f, op=ALU.mult)
        ps_s1 = pp_tile([1, BL], "ps_s1")
        ps_s2 = pp_tile([1, BL], "ps_s2")
        for hc in range(NH):
            sl = slice(hc * 512, (hc + 1) * 512)
            nc.tensor.matmul(ps_s1[:, sl], ones_col, h1f[:, sl],
                             start=True, stop=True, skip_group_check=True)
            nc.tensor.matmul(ps_s2[:, sl], ones_col, sqh[:, sl],
                             start=True, stop=True, skip_group_check=True)
        nmu_h = singles.tile([1, BL], f32, tag="nmu_h", name="nmu_h")
        nc.vector.tensor_scalar_mul(out=nmu_h, in0=ps_s1, scalar1=-1.0 / H)
        musq_h = singles.tile([1, BL], f32, tag="musq", name="musq_h")
        nc.vector.tensor_tensor(out=musq_h, in0=nmu_h, in1=nmu_h, op=ALU.mult)
        v_h = singles.tile([1, BL], f32, tag="v_h", name="v_h")
        nc.vector.tensor_scalar_mul(out=v_h, in0=ps_s2, scalar1=1.0 / H)
        nc.vector.tensor_sub(out=v_h, in0=v_h, in1=musq_h)
        nc.scalar.activation(out=v_h, in_=v_h, func=AF.Sqrt,
                             bias=eps_col[0:1], scale=1.0)
        nc.vector.reciprocal(out=v_h, in_=v_h)
        hstat_dram = dpool.tile([2, BL], f32)
        nc.sync.dma_start(out=hstat_dram[0:1, :], in_=nmu_h)
        nc.sync.dma_start(out=hstat_dram[1:2, :], in_=v_h)
        nmbc = trans.tile([H, BL], f32, tag="u", name="nmbc")
        nc.gpsimd.dma_start(out=nmbc, in_=hstat_dram[0:1, :].to_broadcast([H, BL]))
        rhbc = trans.tile([H, BL], f32, tag="sig_i", name="rhbc")
        nc.gpsimd.dma_start(out=rhbc, in_=hstat_dram[1:2, :].to_broadcast([H, BL]))
        t1 = trans.tile([H, BL], f32, tag="tg", name="t1")
        nc.vector.tensor_tensor(out=t1, in0=h1f, in1=nmbc, op=ALU.add)
        t2 = trans.tile([H, BL], f32, tag="sig_o", name="t2")
        nc.vector.tensor_tensor(out=t2, in0=t1, in1=rhbc, op=ALU.mult)
        last = trans.tile([H, BL], f32, tag="u", name="last")
        nc.vector.tensor_scalar(out=last, in0=t2, scalar1=g_ln_c,
                                scalar2=be_ln_c, op0=ALU.mult, op1=ALU.add)
        pd1 = pg_tile([D1, BL], "pd1")
        for hc in range(NH):
            sl = slice(hc * 512, (hc + 1) * 512)
            nc.tensor.matmul(pd1[:, sl], wd1T, last[:, sl], start=True, stop=True,
                             skip_group_check=True)
        d1 = trans.tile([D1, BL], f32, tag="v_", name="d1")
        nc.scalar.activation(out=d1, in_=pd1, func=AF.Relu, bias=b_d1_c, scale=1.0)
        pd2 = pg_tile([D2, BL], "pd2")
        for hc in range(NH):
            sl = slice(hc * 512, (hc + 1) * 512)
            nc.tensor.matmul(pd2[:, sl], wd2T, d1[:, sl], start=True, stop=True,
                             skip_group_check=True)
        d2 = trans.tile([D2, BL], f32, tag="tc_", name="d2")
        nc.scalar.activation(out=d2, in_=pd2, func=AF.Relu, bias=b_d2_c, scale=1.0)
        pd3 = pg_tile([OUT, BL], "pd3")
        for hc in range(NH):
            sl = slice(hc * 512, (hc + 1) * 512)
            nc.tensor.matmul(pd3[:, sl], wd3T, d2[:, sl], start=True, stop=True,
                             skip_group_check=True)
        o3 = trans.tile([OUT, BL], f32, tag="sig_f", name="o3")
        nc.scalar.activation(out=o3, in_=pd3, func=AF.Identity, bias=b_d3_c,
                             scale=1.0)
        outT = singles.tile([128, QB, OUT], f32)
        for q in range(QB):
            pot = px_tile([128, OUT], "pot")
            nc.tensor.transpose(pot, o3[:, q * 128:(q + 1) * 128],
                                ident[:OUT, :OUT])
            nc.vector.tensor_copy(out=outT[:, q, :], in_=pot)
        nc.sync.dma_start(
            out=out_d[:, :].rearrange("(q p) c -> p q c", p=128),
            in_=outT)
    return nc


_CACHE = {}


def _get_runner():
    if "runner" in _CACHE:
        return _CACHE["runner"]
    import jax
    from jax.sharding import Mesh, PartitionSpec
    from jax.experimental.shard_map import shard_map
    import concourse.bacc as bacc
    import concourse.mybir as mybir
    from concourse.bass2jax import install_neuronx_cc_hook, _bass_exec_p, \
        partition_id_tensor

    nc = bacc.Bacc()
    _build(nc)
    nc.compile()
    install_neuronx_cc_hook()

    partition_name = nc.partition_id_tensor.name if nc.partition_id_tensor else None
    in_names, out_names, out_avals, zero_outs = [], [], [], []
    for alloc in nc.m.functions[0].allocations:
        if not isinstance(alloc, mybir.MemoryLocationSet):
            continue
        name = alloc.memorylocations[0].name
        if alloc.kind == "ExternalInput":
            if name != partition_name:
                in_names.append(name)
        elif alloc.kind == "ExternalOutput":
            out_names.append(name)
            shape = tuple(alloc.tensor_shape)
            dtype = mybir.dt.np(alloc.dtype)
            out_avals.append(jax.core.ShapedArray(shape, dtype))
            zero_outs.append(np.zeros(shape, dtype))
    n_params = len(in_names)
    all_in_names = in_names + out_names + ([partition_name] if partition_name else [])

    def _body(*args):
        operands = list(args)
        if partition_name is not None:
            operands.append(partition_id_tensor())
        outs = _bass_exec_p.bind(
            *operands,
            out_avals=tuple(out_avals),
            in_names=tuple(all_in_names),
            out_names=tuple(out_names),
            lowering_input_output_aliases=(),
            sim_require_finite=False,
            sim_require_nnan=False,
            nc=nc,
        )
        return tuple(outs)

    devices = jax.devices()[:NCORES]
    mesh = Mesh(np.asarray(devices), ("core",))
    in_specs = (PartitionSpec("core"),) * (n_params + len(out_names))
    out_specs = (PartitionSpec("core"),) * len(out_names)
    sharded = jax.jit(
        shard_map(_body, mesh=mesh, in_specs=in_specs, out_specs=out_specs,
                  check_rep=False),
        keep_unused=True)
    _CACHE["runner"] = (sharded, in_names, out_names, zero_outs)
    return _CACHE["runner"]


def kernel(**inputs) -> np.ndarray:
    sharded, in_names, out_names, zero_outs = _get_runner()
    inp = {k: np.ascontiguousarray(np.asarray(v), dtype=np.float32)
           for k, v in inputs.items()}

    def core_val(name, ci):
        if name == "x":
            return inp["x"][ci * BL:(ci + 1) * BL]
        return inp[name]

    concat_in = [
        np.concatenate([core_val(n, ci) for ci in range(NCORES)], axis=0)
        for n in in_names
    ]
    concat_zeros = [
        np.zeros((NCORES * z.shape[0], *z.shape[1:]), z.dtype) for z in zero_outs
    ]
    import jax
    out_arrs = sharded(*concat_in, *concat_zeros)
    jax.block_until_ready(out_arrs)
    oi = out_names.index("out")
    full = np.asarray(out_arrs[oi]).reshape(B, OUT)
    return full.astype(np.float32)



# revision 35
# speedup vs baseline: 1.5356x; 1.5356x over previous
"""DepletionLSTM Trainium2 kernel (v2: composed input weights + fp16).

Self-contained: builds a Bass/Tile kernel for the 2-layer-LSTM network,
shards the batch over 8 NeuronCores (pure data parallelism), runs via
PJRT/axon, returns the full [8192, 30] float32 output.

Strategy (per core, 1024 batch):
- LN stats prepass in [T=90 partitions, batch] layout (quadratic-form
  identity, as before) produces rstd r and -mu*r per (t,b).
- The layer-0 input projection W_in plus the LN scale/shift are COMPOSED
  into the layer-0 gate weights at setup:
    A7_g  = (Wih0 diag(g_in)) @ W_in          [128, 7]  per gate
    B2_g  = (Wih0 diag(g_in)) @ [b_in | 1]    [128, 2]  per gate
  so per step the layer-0 gates are
    A7_g @ (x_t^T * r) + B2_g @ [r; -mu*r] + Whh0_g @ h_{t-1}
  and the per-step [H,H] input projection + PSUM evacuation disappear.
- Per t prepass (pipelined LA steps ahead of the recurrence): one batched
  PE transpose of x_t into 7-row bands [56, 128], one DVE multiply by the
  DMA-broadcast rstd producing the fp16 matmul operand, one small DMA for
  the [2, 1024] rank-2 rows.
- All matmul operands fp16 (1 col/cycle on PE); gate activations fp16 out;
  cell ops on DVE with fp16 inputs (2x DVE mode where operands allow);
  cell state c kept in fp32 for accuracy.
- ACT engine is the bottleneck (10 activations/step of [128,1024]); the
  two layers' emission is interleaved (all 8 gate sigmoids before the two
  tanh(c)) so ACT never stalls on the DVE cell chain.

PSUM: tag "pg" 3x[128,1024] f32 (6 banks) for gates/head, tag "px"
2x[128,128] (2 banks) for transposes/setup.
"""
import sys
sys.path.insert(0, '/opt/trn_rl_repo')

import numpy as np

B, T, F, H, D1, D2, OUT = 8192, 90, 7, 128, 128, 64, 30
NCORES = 8
BL = B // NCORES
G4 = 4 * H
NH = BL // 512
QB = BL // 128
EPS = 1e-5
LA = 3  # prepass lookahead (steps)


def _build(nc, T_steps=T, dbg=False):
    import concourse.tile as tile
    from concourse import mybir
    from concourse.masks import make_identity

    f32 = mybir.dt.float32
    f32r = mybir.dt.float32r
    fp16 = mybir.dt.float16
    AF = mybir.ActivationFunctionType
    ALU = mybir.AluOpType

    TS = T_steps

    # ---------------- DRAM I/O ----------------
    x_d = nc.dram_tensor("x", [BL, T, F], f32, kind="ExternalInput")
    W_in_d = nc.dram_tensor("W_in", [H, F], f32, kind="ExternalInput")
    b_in_d = nc.dram_tensor("b_in", [H], f32, kind="ExternalInput")
    g_in_d = nc.dram_tensor("g_in", [H], f32, kind="ExternalInput")
    be_in_d = nc.dram_tensor("be_in", [H], f32, kind="ExternalInput")
    Wih_d = [nc.dram_tensor("Wih0", [G4, H], f32, kind="ExternalInput"),
             nc.dram_tensor("Wih1", [G4, H], f32, kind="ExternalInput")]
    Whh_d = [nc.dram_tensor("Whh0", [G4, H], f32, kind="ExternalInput"),
             nc.dram_tensor("Whh1", [G4, H], f32, kind="ExternalInput")]
    bih_d = [nc.dram_tensor("bih0", [G4], f32, kind="ExternalInput"),
             nc.dram_tensor("bih1", [G4], f32, kind="ExternalInput")]
    bhh_d = [nc.dram_tensor("bhh0", [G4], f32, kind="ExternalInput"),
             nc.dram_tensor("bhh1", [G4], f32, kind="ExternalInput")]
    g_ln_d = nc.dram_tensor("g_ln", [H], f32, kind="ExternalInput")
    be_ln_d = nc.dram_tensor("be_ln", [H], f32, kind="ExternalInput")
    W_d1_d = nc.dram_tensor("W_d1", [D1, H], f32, kind="ExternalInput")
    b_d1_d = nc.dram_tensor("b_d1", [D1], f32, kind="ExternalInput")
    W_d2_d = nc.dram_tensor("W_d2", [D2, D1], f32, kind="ExternalInput")
    b_d2_d = nc.dram_tensor("b_d2", [D2], f32, kind="ExternalInput")
    W_d3_d = nc.dram_tensor("W_d3", [OUT, D2], f32, kind="ExternalInput")
    b_d3_d = nc.dram_tensor("b_d3", [OUT], f32, kind="ExternalInput")
    out_d = nc.dram_tensor("out", [BL, OUT], f32, kind="ExternalOutput")
    if dbg:
        dbg_xbr = nc.dram_tensor("dbg_xbr", [F, BL], f32, kind="ExternalOutput")
        dbg_rn = nc.dram_tensor("dbg_rn", [2, BL], f32, kind="ExternalOutput")
        dbg_h0 = nc.dram_tensor("dbg_h0", [H, BL], f32, kind="ExternalOutput")
        dbg_c0 = nc.dram_tensor("dbg_c0", [H, BL], f32, kind="ExternalOutput")
        dbg_g0 = nc.dram_tensor("dbg_g0", [H, BL], f32, kind="ExternalOutput")

    import contextlib
    with tile.TileContext(nc) as tc, contextlib.ExitStack() as ctx:
        singles = ctx.enter_context(tc.tile_pool(name="singles", bufs=1))
        trans = ctx.enter_context(tc.tile_pool(name="trans", bufs=2))
        small = ctx.enter_context(tc.tile_pool(name="small", bufs=1))
        feed = ctx.enter_context(tc.tile_pool(name="feed", bufs=LA + 1))
        ps_pg = ctx.enter_context(tc.tile_pool(name="ps_pg", bufs=3, space="PSUM"))
        ps_px = ctx.enter_context(tc.tile_pool(name="ps_px", bufs=2, space="PSUM"))
        dpool = ctx.enter_context(tc.tile_pool(name="dpool", bufs=1, space="DRAM"))

        def pg_tile(shape, name):
            return ps_pg.tile(shape, f32, tag="pg", name=name)

        def px_tile(shape, name):
            return ps_px.tile(shape, f32, tag="pxt", name=name)

        # ---------------- constants ----------------
        ident = singles.tile([128, 128], f32)
        make_identity(nc, ident)
        ones_row = singles.tile([1, 512], f32)
        nc.vector.memset(ones_row, 1.0)
        ones_col = singles.tile([128, 1], f32)
        nc.vector.memset(ones_col, 1.0)
        eps_col = singles.tile([T, 1], f32)
        nc.vector.memset(eps_col, EPS)

        def load_col(dram_vec, n, name):
            t_ = singles.tile([n, 1], f32, name=name, tag=name)
            nc.sync.dma_start(out=t_, in_=dram_vec[:].rearrange("(p o) -> p o", o=1))
            return t_

        # ---------------- x loads (issued first: everything gates on them) ----
        # xs[p, q, t, f] = x[128q+p, t, f]; contiguous (t,f) runs -> cheap DMA
        xs = singles.tile([128, QB, T, F], f32)
        nc.sync.dma_start(
            out=xs, in_=x_d[:, :, :].rearrange("(q p) t f -> p q t f", p=128))
        # fp16 copy: halves the per-step PE transpose cost (1.0 vs 2.0 c/row)
        xs16 = singles.tile([128, QB, T, F], fp16)
        nc.vector.tensor_copy(out=xs16, in_=xs)
        ident16 = singles.tile([128, 128], fp16)
        nc.vector.tensor_copy(out=ident16, in_=ident)

        g_in_c = load_col(g_in_d, H, "g_in_c")
        be_in_c = load_col(be_in_d, H, "be_in_c")
        b_in_c = load_col(b_in_d, H, "b_in_c")
        g_ln_c = load_col(g_ln_d, H, "g_ln_c")
        be_ln_c = load_col(be_ln_d, H, "be_ln_c")
        b_d1_c = load_col(b_d1_d, D1, "b_d1_c")
        b_d2_c = load_col(b_d2_d, D2, "b_d2_c")
        b_d3_c = load_col(b_d3_d, OUT, "b_d3_c")

        # bn1T [H, 2]: col0 = b_in, col1 = ones (for the rank-2 LN rows)
        bn1T = singles.tile([H, 2], f32)
        nc.vector.tensor_copy(out=bn1T[:, 0:1], in_=b_in_c)
        nc.vector.tensor_copy(out=bn1T[:, 1:2], in_=ones_col)

        # stats layout x16_tm[t, q, f, b] built from xs16 via PE transposes
        # (cheaper than the 92k-descriptor strided DMA reload of x)
        x16_tm = singles.tile([T, QB, F, 128], fp16)
        for q in range(QB):
            ptq = ps_px.tile([T, F, 128], fp16, tag="pxt", name="ptq")
            for fi in range(F):
                nc.tensor.transpose(ptq[:TS, fi, :], xs16[:, q, :TS, fi],
                                    ident16)
            nc.vector.tensor_copy(out=x16_tm[:TS, q, :, :], in_=ptq[:TS])

        # ---------------- weights: load + PE-transpose ----------------
        w_in_raw = singles.tile([H, F], f32)
        nc.sync.dma_start(out=w_in_raw, in_=W_in_d[:, :])

        # transposed gate weights; layer-0 input weights only needed to
        # compose A7/B2 (f32 scratch), recurrent + layer-1 kept fp16.
        wihT0g = singles.tile([H, 4, H], f32)   # gamma-scaled Wih0^T
        wihT0f = singles.tile([H, 4, H], f32)   # unscaled (for beta fold)
        whhT = [singles.tile([H, 4, H], fp16, name="whhT0", tag="whhT0"),
                singles.tile([H, 4, H], fp16, name="whhT1", tag="whhT1")]
        wihT1 = singles.tile([H, 4, H], fp16)
        wih_raw = [trans.tile([H, 4, H], f32, tag="wraw", name=f"wih_raw{L}")
                   for L in range(2)]
        whh_raw = [trans.tile([H, 4, H], f32, tag="hraw", name=f"whh_raw{L}")
                   for L in range(2)]
        for L in range(2):
            nc.sync.dma_start(
                out=wih_raw[L],
                in_=Wih_d[L][:, :].rearrange("(c p) h -> p c h", p=H))
            nc.sync.dma_start(
                out=whh_raw[L],
                in_=Whh_d[L][:, :].rearrange("(c p) h -> p c h", p=H))
        for L in range(2):
            for cc in range(4):
                pt_w = px_tile([H, H], "tr_ps_w")
                nc.tensor.transpose(pt_w, wih_raw[L][:, cc, :], ident)
                if L == 0:
                    nc.scalar.copy(out=wihT0f[:, cc, :], in_=pt_w)
                    nc.scalar.mul(out=wihT0g[:, cc, :], in_=pt_w, mul=g_in_c)
                else:
                    nc.scalar.copy(out=wihT1[:, cc, :], in_=pt_w)
                pt_h = px_tile([H, H], "tr_ps_w")
                nc.tensor.transpose(pt_h, whh_raw[L][:, cc, :], ident)
                nc.scalar.copy(out=whhT[L][:, cc, :], in_=pt_h)

        # composed layer-0 input weights (fp16): A7 = Wih0g @ W_in,
        # B2 = Wih0g @ [b_in | 1] (separate tiles: compute engines may only
        # address partition ranges starting at 0/32/64)
        A7 = singles.tile([F, 4, H], fp16)
        B2 = singles.tile([2, 4, H], fp16)
        for cc in range(4):
            pa = px_tile([F, H], "pa")
            nc.tensor.matmul(pa, w_in_raw, wihT0g[:, cc, :], start=True, stop=True)
            nc.scalar.copy(out=A7[:, cc, :], in_=pa)
            pb2 = px_tile([2, H], "pa")
            nc.tensor.matmul(pb2, bn1T, wihT0g[:, cc, :], start=True, stop=True)
            nc.scalar.copy(out=B2[:, cc, :], in_=pb2)

        # gate biases beff[L] [128, 4]; layer-0 gets the beta fold Wih0 @ be_in
        beff = []
        for L in range(2):
            bt_ = singles.tile([H, 4], f32, name=f"beff{L}", tag=f"beff{L}")
            bih_sb = small.tile([H, 4], f32, tag="bload", name="bih_sb")
            nc.sync.dma_start(out=bih_sb,
                              in_=bih_d[L][:].rearrange("(c p) -> p c", p=H))
            bhh_sb = small.tile([H, 4], f32, tag="bload2", name="bhh_sb")
            nc.sync.dma_start(out=bhh_sb,
                              in_=bhh_d[L][:].rearrange("(c p) -> p c", p=H))
            nc.vector.tensor_add(out=bt_, in0=bih_sb, in1=bhh_sb)
            beff.append(bt_)
        for cc in range(4):
            pb = px_tile([H, 1], "pa")
            nc.tensor.matmul(pb, wihT0f[:, cc, :], be_in_c, start=True, stop=True)
            nc.vector.tensor_add(out=beff[0][:, cc:cc + 1],
                                 in0=beff[0][:, cc:cc + 1], in1=pb)

        # head weights
        def transpose_to(dst, src_ap, p, fdim):
            pt = px_tile([fdim, p], "pa")
            nc.tensor.transpose(pt, src_ap, ident[:p, :p])
            nc.scalar.copy(out=dst, in_=pt)

        wd1T = singles.tile([H, D1], fp16)
        wd1_raw = trans.tile([D1, H], f32, tag="wraw", name="wd1_raw")
        nc.sync.dma_start(out=wd1_raw, in_=W_d1_d[:, :])
        transpose_to(wd1T, wd1_raw, D1, H)
        wd2T = singles.tile([D1, D2], fp16)
        wd2_raw = trans.tile([D2, D1], f32, tag="hraw", name="wd2_raw")
        nc.sync.dma_start(out=wd2_raw, in_=W_d2_d[:, :])
        transpose_to(wd2T, wd2_raw, D2, D1)
        wd3T = singles.tile([D2, OUT], fp16)
        wd3_raw = trans.tile([OUT, D2], f32, tag="wraw", name="wd3_raw")
        nc.sync.dma_start(out=wd3_raw, in_=W_d3_d[:, :])
        transpose_to(wd3T, wd3_raw, OUT, D2)
        ones16 = singles.tile([128, 1], fp16)
        nc.vector.tensor_copy(out=ones16, in_=ones_col)

        # ---------------- prepass: LN stats in [T, BL] layout ----------------
        # p' = W_in x + b_in per (h | b,t); over h:
        #   sum p'   = wsum . x + bsum
        #   sum p'^2 = x^T M x + 2 l^T x + c0,  M = W^T W, l = W^T b, c0=|b|^2
        p_m = px_tile([F, F], "pa")
        nc.tensor.matmul(p_m, w_in_raw, w_in_raw, start=True, stop=True)
        m_sb = small.tile([F, F], f32, tag="m_sb", name="m_sb")
        nc.vector.tensor_copy(out=m_sb, in_=p_m)
        p_ws = px_tile([1, F], "pa")
        nc.tensor.matmul(p_ws, ones_col, w_in_raw, start=True, stop=True)
        ws_sb = small.tile([1, F], f32, tag="ws_sb", name="ws_sb")
        nc.vector.tensor_copy(out=ws_sb, in_=p_ws)
        p_l = px_tile([1, F], "pa")
        nc.tensor.matmul(p_l, b_in_c, w_in_raw, start=True, stop=True)
        l_sb = small.tile([1, F], f32, tag="l_sb", name="l_sb")
        nc.vector.tensor_copy(out=l_sb, in_=p_l)
        p_sc = px_tile([1, 2], "pa")
        nc.tensor.matmul(p_sc[:, 0:1], b_in_c, b_in_c, start=True, stop=False,
                         skip_group_check=True)
        nc.tensor.matmul(p_sc[:, 1:2], ones_col, b_in_c, start=False, stop=True,
                         skip_group_check=True)
        sc_sb = small.tile([1, 2], f32, tag="sc_sb", name="sc_sb")
        nc.vector.tensor_copy(out=sc_sb, in_=p_sc)
        # stage stat constants to DRAM, then one partition-broadcast back
        NST = F * F + 2 * F + 2
        stat_dram = dpool.tile([1, NST], f32)
        nc.sync.dma_start(
            out=stat_dram[0:1, 0:F * F].rearrange("o (a b) -> (o a) b", a=F),
            in_=m_sb)
        nc.sync.dma_start(out=stat_dram[0:1, F * F:F * F + F], in_=ws_sb)
        nc.sync.dma_start(out=stat_dram[0:1, F * F + F:F * F + 2 * F], in_=l_sb)
        nc.sync.dma_start(out=stat_dram[0:1, F * F + 2 * F:NST], in_=sc_sb)
        statbc = singles.tile([T, NST], f32)
        nc.gpsimd.dma_start(out=statbc, in_=stat_dram[0:1, :].to_broadcast([T, NST]))
        mbc = statbc[:, 0:F * F]
        wbc = statbc[:, F * F:F * F + F]
        lbc = statbc[:, F * F + F:F * F + 2 * F]
        scbc = statbc[:, F * F + 2 * F:NST]

        def xf(fi):
            return x16_tm[:TS, :, fi, :]

        def g3(ap):
            return ap.rearrange("t (q b) -> t q b", b=128)

        # fp16 products + pairwise quadratic form (ts 4x / TT 2x DVE modes;
        # the 3-operand scalar_tensor_tensor has no fast mode so avoid it)
        nmu_all = singles.tile([T, BL], f32)
        r_all = singles.tile([T, BL], f32)

        def dot_chain(dst, coef_col):
            # dst = sum_f coef[f] * x_f  via 7 scaled products + 6 adds
            pa_ = trans.tile([T, BL], fp16, tag="tg", name="st_pa")
            pb_ = trans.tile([T, BL], fp16, tag="sig_o", name="st_pb")
            nc.vector.tensor_scalar_mul(out=g3(pa_[:TS]), in0=xf(0),
                                        scalar1=coef_col(0))
            for fi in range(1, F):
                nc.vector.tensor_scalar_mul(out=g3(pb_[:TS]), in0=xf(fi),
                                            scalar1=coef_col(fi))
                nc.vector.tensor_add(out=dst[:TS] if fi == F - 1 else pa_[:TS],
                                     in0=pa_[:TS], in1=pb_[:TS])

        acc = trans.tile([T, BL], fp16, tag="sig_i", name="st_acc")
        dot_chain(acc, lambda fi: wbc[:TS, fi:fi + 1])
        # nmu = -(acc + bsum)/H
        nc.vector.tensor_scalar(out=nmu_all[:TS], in0=acc[:TS],
                                scalar1=scbc[:TS, 1:2], scalar2=-1.0 / H,
                                op0=ALU.add, op1=ALU.mult)
        lin = trans.tile([T, BL], fp16, tag="u", name="st_lin")
        dot_chain(lin, lambda fi: lbc[:TS, fi:fi + 1])
        # qform = sum_{i<=j} c_ij x_i x_j, c_ij = M_ii or 2 M_ij; fold the
        # lin term in as a virtual pair via the accumulate chain
        qacc = trans.tile([T, BL], fp16, tag="sig_f", name="st_qacc")
        ts_ = trans.tile([T, BL], fp16, tag="tg", name="st_ts")
        tp_ = trans.tile([T, BL], fp16, tag="sig_o", name="st_tp")
        first = True
        for fi in range(F):
            for fj in range(fi, F):
                mcol = mbc[:TS, fi * F + fj:fi * F + fj + 1]
                if fi == fj:
                    nc.vector.tensor_scalar_mul(out=g3(ts_[:TS]), in0=xf(fj),
                                                scalar1=mcol)
                else:
                    nc.vector.tensor_scalar(out=g3(ts_[:TS]), in0=xf(fj),
                                            scalar1=mcol, scalar2=2.0,
                                            op0=ALU.mult, op1=ALU.mult)
                nc.vector.tensor_tensor(
                    out=g3(tp_[:TS]) if not first else g3(qacc[:TS]),
                    in0=g3(ts_[:TS]), in1=xf(fi), op=ALU.mult)
                if not first:
                    nc.vector.tensor_add(out=qacc[:TS], in0=qacc[:TS],
                                         in1=tp_[:TS])
                first = False
        # qacc += 2 * lin
        nc.vector.tensor_scalar_mul(out=lin[:TS], in0=lin[:TS], scalar1=2.0)
        nc.vector.tensor_add(out=qacc[:TS], in0=qacc[:TS], in1=lin[:TS])
        # var = (q + c0)/H - mu^2 ; r = 1/sqrt(var+eps)  (f32 tail)
        var32 = trans.tile([T, BL], f32, tag="v_", name="var32")
        nc.vector.tensor_scalar(out=var32[:TS], in0=qacc[:TS],
                                scalar1=scbc[:TS, 0:1], scalar2=1.0 / H,
                                op0=ALU.add, op1=ALU.mult)
        musq = trans.tile([T, BL], f32, tag="v32", name="musq")
        nc.vector.tensor_tensor(out=musq[:TS], in0=nmu_all[:TS],
                                in1=nmu_all[:TS], op=ALU.mult)
        nc.vector.tensor_sub(out=var32[:TS], in0=var32[:TS], in1=musq[:TS])
        nc.scalar.activation(out=r_all[:TS], in_=var32[:TS], func=AF.Sqrt,
                             bias=eps_col[:TS], scale=1.0)
        nc.vector.reciprocal(out=r_all[:TS], in_=r_all[:TS])
        # fp16 rows for the prepass broadcast + rank-2 matmul rows
        r16 = trans.tile([T, BL], fp16, tag="tg", name="r16")
        nc.vector.tensor_copy(out=r16[:TS], in_=r_all[:TS])
        nmr16 = trans.tile([T, BL], fp16, tag="sig_o", name="nmr16")
        nc.vector.tensor_tensor(out=nmr16[:TS], in0=nmu_all[:TS],
                                in1=r_all[:TS], op=ALU.mult)
        # DRAM staging: fp16 [r; -mu*r] rows
        rnm16_dram = dpool.tile([2, T, BL], fp16)
        nc.sync.dma_start(out=rnm16_dram[0, :TS], in_=r16[:TS])
        nc.sync.dma_start(out=rnm16_dram[1, :TS], in_=nmr16[:TS])

        # ---------------- states ----------------
        h1 = singles.tile([H, BL], fp16, name="h1", tag="h1")
        c = [singles.tile([H, BL], f32, name="c0", tag="c0"),
             singles.tile([H, BL], f32, name="c1", tag="c1")]
        zinit = trans.tile([H, BL], f32, tag="u", name="zinit")
        nc.vector.memset(zinit, 0.0)
        h0_prev = trans.tile([H, BL], fp16, tag="h0", name="h0_init")
        nc.vector.tensor_copy(out=h0_prev, in_=zinit)
        nc.vector.tensor_copy(out=h1, in_=zinit)
        for L in range(2):
            nc.vector.memset(c[L], 0.0)

        # ---------------- per-step prepass ----------------
        def prepass(t):
            # feature-major x_t: 8 PE transposes into [7, 1024] column blocks
            px = ps_px.tile([F, BL], fp16, tag="pxt", name="pxt_main")
            for q in range(QB):
                nc.tensor.transpose(px[:, q * 128:(q + 1) * 128],
                                    xs16[:, q, t, :], ident16)
            # rstd broadcast over the 7 feature rows (fp16 -> 2x DVE mode)
            rb = feed.tile([F, BL], fp16, tag="rb", name="rb", bufs=2)
            nc.gpsimd.dma_start(
                out=rb, in_=rnm16_dram[0, t:t + 1, :].to_broadcast([F, BL]))
            xbr = feed.tile([F, BL], fp16, tag="xbr", name="xbr")
            nc.vector.tensor_tensor(out=xbr, in0=px, in1=rb, op=ALU.mult)
            # rank-2 rows [r; -mu*r]
            rn = feed.tile([2, BL], fp16, tag="rn", name="rn")
            nc.gpsimd.dma_start(out=rn, in_=rnm16_dram[:, t, :])
            return xbr, rn

        feed_bufs = {}
        for t in range(min(LA, TS)):
            feed_bufs[t] = prepass(t)

        # ---------------- main loop ----------------
        sig_funcs = [AF.Sigmoid, AF.Sigmoid, AF.Tanh, AF.Sigmoid]

        def gate_l0(gc, xbr, rn, hprev):
            pg = pg_tile([H, BL], "pg_gates")
            for hc in range(NH):
                sl = slice(hc * 512, (hc + 1) * 512)
                nc.tensor.matmul(pg[:, sl], A7[:, gc, :], xbr[:, sl],
                                 start=True, stop=False,
                                 skip_group_check=True)
                nc.tensor.matmul(pg[:, sl], B2[:, gc, :], rn[:, sl],
                                 start=False, stop=False,
                                 skip_group_check=True)
                nc.tensor.matmul(pg[:, sl], whhT[0][:, gc, :], hprev[:, sl],
                                 start=False, stop=(hc == NH - 1),
                                 skip_group_check=True)
            g_out = trans.tile([H, BL], fp16, tag=f"g{gc}", name=f"g0_{gc}")
            nc.scalar.activation(out=g_out, in_=pg, func=sig_funcs[gc],
                                 bias=beff[0][:, gc:gc + 1], scale=1.0)
            return g_out

        def gate_l1_mm(gc, hprev):
            pg = pg_tile([H, BL], "pg_gates")
            for hc in range(NH):
                sl = slice(hc * 512, (hc + 1) * 512)
                nc.tensor.matmul(pg[:, sl], wihT1[:, gc, :], hprev[:, sl],
                                 start=True, stop=False,
                                 skip_group_check=True)
                nc.tensor.matmul(pg[:, sl], whhT[1][:, gc, :], h1[:, sl],
                                 start=False, stop=True,
                                 skip_group_check=True)
            return pg

        def gate_l1_act(gc, pg):
            g_out = trans.tile([H, BL], fp16, tag=f"g{gc}", name=f"g1_{gc}")
            nc.scalar.activation(out=g_out, in_=pg, func=sig_funcs[gc],
                                 bias=beff[1][:, gc:gc + 1], scale=1.0)
            return g_out

        def iteration(t, h0_prev):
            """One fused iteration: layer-0 step t plus layer-1 step t-1
            (g1 state carried in nonlocal l1g).  Emission order keeps the
            ACT stream dense: sigma0 x4, sigma1 i/f, tanh_c0, sigma1 g/o,
            tanh_c1."""
            xbr_t, rn_t = feed_bufs.pop(t)
            # L0 gates (PE + ACT)
            g0 = [gate_l0(gc, xbr_t, rn_t, h0_prev) for gc in range(4)]
            # L1 matmuls for step t-1 + first two activations
            pg1 = [gate_l1_mm(gc, h0_prev) for gc in range(4)] if t > 0 else None
            g1 = [gate_l1_act(gc, pg1[gc]) for gc in range(2)] if t > 0 else None
            # L0 cell chain: u, v, c
            u0 = trans.tile([H, BL], fp16, tag="u16", name="u0")
            nc.vector.tensor_tensor(out=u0, in0=g0[0], in1=g0[2], op=ALU.mult)
            v0 = trans.tile([H, BL], f32, tag="v32", name="v0")
            nc.vector.tensor_tensor(out=v0, in0=g0[1], in1=c[0], op=ALU.mult)
            nc.vector.tensor_add(out=c[0], in0=u0, in1=v0)
            tc0 = trans.tile([H, BL], fp16, tag="tc16", name="tc0")
            nc.scalar.activation(out=tc0, in_=c[0], func=AF.Tanh, scale=1.0)
            h0_new = trans.tile([H, BL], fp16, tag="h0", name="h0_new")
            nc.vector.tensor_tensor(out=h0_new, in0=g0[3], in1=tc0, op=ALU.mult)
            # prepass for t+LA behind the gate matmuls
            if t + LA < TS:
                feed_bufs[t + LA] = prepass(t + LA)
            if t > 0:
                g1 = g1 + [gate_l1_act(gc, pg1[gc]) for gc in range(2, 4)]
                u1 = trans.tile([H, BL], fp16, tag="u16", name="u1")
                nc.vector.tensor_tensor(out=u1, in0=g1[0], in1=g1[2],
                                        op=ALU.mult)
                v1 = trans.tile([H, BL], f32, tag="v32", name="v1")
                nc.vector.tensor_tensor(out=v1, in0=g1[1], in1=c[1],
                                        op=ALU.mult)
                nc.vector.tensor_add(out=c[1], in0=u1, in1=v1)
                tc1 = trans.tile([H, BL], fp16, tag="tc16", name="tc1")
                nc.scalar.activation(out=tc1, in_=c[1], func=AF.Tanh, scale=1.0)
                nc.vector.tensor_tensor(out=h1, in0=g1[3], in1=tc1,
                                        op=ALU.mult)
            if dbg and t == 0:
                dxbr = trans.tile([F, BL], f32, tag="u16", name="dxbr")
                nc.vector.tensor_copy(out=dxbr, in_=xbr_t)
                nc.sync.dma_start(out=dbg_xbr[:, :], in_=dxbr)
                drn = trans.tile([2, BL], f32, tag="v32", name="drn")
                nc.vector.tensor_copy(out=drn, in_=rn_t)
                nc.sync.dma_start(out=dbg_rn[:, :], in_=drn)
                dh0 = trans.tile([H, BL], f32, tag="v32", name="dh0")
                nc.vector.tensor_copy(out=dh0, in_=h0_new)
                nc.sync.dma_start(out=dbg_h0[:, :], in_=dh0)
                nc.sync.dma_start(out=dbg_c0[:, :], in_=c[0])
                dg0 = trans.tile([H, BL], f32, tag="u16", name="dg0")
                nc.vector.tensor_copy(out=dg0, in_=g0[0])
                nc.sync.dma_start(out=dbg_g0[:, :], in_=dg0)
            return h0_new

        for t in range(TS):
            h0_prev = iteration(t, h0_prev)
        # final layer-1 step (t = TS-1)
        pg1 = [gate_l1_mm(gc, h0_prev) for gc in range(4)]
        g1 = [gate_l1_act(gc, pg1[gc]) for gc in range(4)]
        u1 = trans.tile([H, BL], fp16, tag="u16", name="u1f")
        nc.vector.tensor_tensor(out=u1, in0=g1[0], in1=g1[2], op=ALU.mult)
        v1 = trans.tile([H, BL], f32, tag="v32", name="v1f")
        nc.vector.tensor_tensor(out=v1, in0=g1[1], in1=c[1], op=ALU.mult)
        nc.vector.tensor_add(out=c[1], in0=u1, in1=v1)
        tc1 = trans.tile([H, BL], fp16, tag="tc16", name="tc1f")
        nc.scalar.activation(out=tc1, in_=c[1], func=AF.Tanh, scale=1.0)
        nc.vector.tensor_tensor(out=h1, in0=g1[3], in1=tc1, op=ALU.mult)

        # ---------------- head ----------------
        # LN sums per q-block -> [8, 128] combine ops (cheap free size)
        sqh = trans.tile([H, BL], fp16, tag="g0", name="sqh")
        nc.vector.tensor_tensor(out=sqh, in0=h1, in1=h1, op=ALU.mult)
        ps_s1 = pg_tile([1, BL], "ps_s1")
        ps_s2 = pg_tile([1, BL], "ps_s2")
        for hc in range(NH):
            sl = slice(hc * 512, (hc + 1) * 512)
            nc.tensor.matmul(ps_s1[:, sl], ones16, h1[:, sl],
                             start=True, stop=True, skip_group_check=True)
            nc.tensor.matmul(ps_s2[:, sl], ones16, sqh[:, sl],
                             start=True, stop=True, skip_group_check=True)
        nmu_h = small.tile([1, BL], fp16, tag="nmu_h", name="nmu_h")
        nc.vector.tensor_scalar_mul(out=nmu_h, in0=ps_s1, scalar1=-1.0 / H)
        musq_h = small.tile([1, BL], fp16, tag="musq", name="musq_h")
        nc.vector.tensor_tensor(out=musq_h, in0=nmu_h, in1=nmu_h, op=ALU.mult)
        v_h = small.tile([1, BL], f32, tag="v_h", name="v_h")
        nc.vector.tensor_scalar_mul(out=v_h, in0=ps_s2, scalar1=1.0 / H)
        nc.vector.tensor_sub(out=v_h, in0=v_h, in1=musq_h)
        nc.scalar.activation(out=v_h, in_=v_h, func=AF.Sqrt,
                             bias=eps_col[0:1], scale=1.0)
        nc.vector.reciprocal(out=v_h, in_=v_h)
        hstat_dram = dpool.tile([1, BL], fp16)
        hstat32_dram = dpool.tile([1, BL], f32)
        nc.sync.dma_start(out=hstat_dram[0:1, :], in_=nmu_h)
        nc.sync.dma_start(out=hstat32_dram[0:1, :], in_=v_h)
        nmbc = trans.tile([H, BL], fp16, tag="g1", name="nmbc")
        nc.gpsimd.dma_start(
            out=nmbc, in_=hstat_dram[0:1, :].to_broadcast([H, BL]))
        rhbc = trans.tile([H, BL], f32, tag="g2", name="rhbc")
        nc.sync.dma_start(
            out=rhbc, in_=hstat32_dram[0:1, :].to_broadcast([H, BL]))
        t1 = trans.tile([H, BL], fp16, tag="g3", name="t1")
        nc.vector.tensor_tensor(out=t1, in0=h1, in1=nmbc, op=ALU.add)
        t2 = trans.tile([H, BL], fp16, tag="g0", name="t2")
        nc.vector.tensor_tensor(out=t2, in0=t1, in1=rhbc, op=ALU.mult)
        last = trans.tile([H, BL], fp16, tag="g1", name="last")
        nc.vector.tensor_scalar(out=last, in0=t2, scalar1=g_ln_c,
                                scalar2=be_ln_c, op0=ALU.mult, op1=ALU.add)
        pd1 = pg_tile([D1, BL], "pd1")
        for hc in range(NH):
            sl = slice(hc * 512, (hc + 1) * 512)
            nc.tensor.matmul(pd1[:, sl], wd1T, last[:, sl],
                             start=True, stop=True, skip_group_check=True)
        d1 = trans.tile([D1, BL], fp16, tag="g2", name="d1")
        nc.scalar.activation(out=d1, in_=pd1, func=AF.Relu, bias=b_d1_c, scale=1.0)
        pd2 = pg_tile([D2, BL], "pd2")
        for hc in range(NH):
            sl = slice(hc * 512, (hc + 1) * 512)
            nc.tensor.matmul(pd2[:, sl], wd2T, d1[:, sl],
                             start=True, stop=True, skip_group_check=True)
        d2 = trans.tile([D2, BL], fp16, tag="g3", name="d2")
        nc.scalar.activation(out=d2, in_=pd2, func=AF.Relu, bias=b_d2_c, scale=1.0)
        pd3 = pg_tile([OUT, BL], "pd3")
        for hc in range(NH):
            sl = slice(hc * 512, (hc + 1) * 512)
            nc.tensor.matmul(pd3[:, sl], wd3T, d2[:, sl],
                             start=True, stop=True, skip_group_check=True)
        o3 = trans.tile([OUT, BL], f32, tag="sig_f", name="o3")
        nc.scalar.activation(out=o3, in_=pd3, func=AF.Identity, bias=b_d3_c,
                             scale=1.0)
        outT = singles.tile([128, QB, OUT], f32)
        for q in range(QB):
            pot = px_tile([128, OUT], "pot")
            nc.tensor.transpose(pot, o3[:, q * 128:(q + 1) * 128],
                                ident[:OUT, :OUT])
            nc.vector.tensor_copy(out=outT[:, q, :], in_=pot)
        nc.sync.dma_start(
            out=out_d[:, :].rearrange("(q p) c -> p q c", p=128),
            in_=outT)
    return nc


_CACHE = {}


def _get_runner():
    if "runner" in _CACHE:
        return _CACHE["runner"]
    import jax
    from jax.sharding import Mesh, PartitionSpec
    from jax.experimental.shard_map import shard_map
    import concourse.bacc as bacc
    import concourse.mybir as mybir
    from concourse.bass2jax import install_neuronx_cc_hook, _bass_exec_p, \
        partition_id_tensor

    nc = bacc.Bacc()
    _build(nc)
    nc.compile()
    install_neuronx_cc_hook()

    partition_name = nc.partition_id_tensor.name if nc.partition_id_tensor else None
    in_names, out_names, out_avals, zero_outs = [], [], [], []
    for alloc in nc.m.functions[0].allocations:
        if not isinstance(alloc, mybir.MemoryLocationSet):
            continue
        name = alloc.memorylocations[0].name
        if alloc.kind == "ExternalInput":
            if name != partition_name:
                in_names.append(name)
        elif alloc.kind == "ExternalOutput":
            out_names.append(name)
            shape = tuple(alloc.tensor_shape)
            dtype = mybir.dt.np(alloc.dtype)
            out_avals.append(jax.core.ShapedArray(shape, dtype))
            zero_outs.append(np.zeros(shape, dtype))
    n_params = len(in_names)
    all_in_names = in_names + out_names + ([partition_name] if partition_name else [])

    def _body(*args):
        operands = list(args)
        if partition_name is not None:
            operands.append(partition_id_tensor())
        outs = _bass_exec_p.bind(
            *operands,
            out_avals=tuple(out_avals),
            in_names=tuple(all_in_names),
            out_names=tuple(out_names),
            lowering_input_output_aliases=(),
            sim_require_finite=False,
            sim_require_nnan=False,
            nc=nc,
        )
        return tuple(outs)

    devices = jax.devices()[:NCORES]
    mesh = Mesh(np.asarray(devices), ("core",))
    in_specs = (PartitionSpec("core"),) * (n_params + len(out_names))
    out_specs = (PartitionSpec("core"),) * len(out_names)
    sharded = jax.jit(
        shard_map(_body, mesh=mesh, in_specs=in_specs, out_specs=out_specs,
                  check_rep=False),
        keep_unused=True)
    _CACHE["runner"] = (sharded, in_names, out_names, zero_outs)
    return _CACHE["runner"]


def kernel(**inputs) -> np.ndarray:
    sharded, in_names, out_names, zero_outs = _get_runner()
    inp = {k: np.ascontiguousarray(np.asarray(v), dtype=np.float32)
           for k, v in inputs.items()}

    def core_val(name, ci):
        if name == "x":
            return inp["x"][ci * BL:(ci + 1) * BL]
        return inp[name]

    concat_in = [
        np.concatenate([core_val(n, ci) for ci in range(NCORES)], axis=0)
        for n in in_names
    ]
    concat_zeros = [
        np.zeros((NCORES * z.shape[0], *z.shape[1:]), z.dtype) for z in zero_outs
    ]
    import jax
    out_arrs = sharded(*concat_in, *concat_zeros)
    jax.block_until_ready(out_arrs)
    oi = out_names.index("out")
    full = np.asarray(out_arrs[oi]).reshape(B, OUT)
    return full.astype(np.float32)


# revision 37
# speedup vs baseline: 1.5619x; 1.0171x over previous
"""DepletionLSTM Trainium2 kernel (v2: composed input weights + fp16).

Self-contained: builds a Bass/Tile kernel for the 2-layer-LSTM network,
shards the batch over 8 NeuronCores (pure data parallelism), runs via
PJRT/axon, returns the full [8192, 30] float32 output.

Strategy (per core, 1024 batch):
- LN stats prepass in [T=90 partitions, batch] layout (quadratic-form
  identity, as before) produces rstd r and -mu*r per (t,b).
- The layer-0 input projection W_in plus the LN scale/shift are COMPOSED
  into the layer-0 gate weights at setup:
    A7_g  = (Wih0 diag(g_in)) @ W_in          [128, 7]  per gate
    B2_g  = (Wih0 diag(g_in)) @ [b_in | 1]    [128, 2]  per gate
  so per step the layer-0 gates are
    A7_g @ (x_t^T * r) + B2_g @ [r; -mu*r] + Whh0_g @ h_{t-1}
  and the per-step [H,H] input projection + PSUM evacuation disappear.
- Per t prepass (pipelined LA steps ahead of the recurrence): one batched
  PE transpose of x_t into 7-row bands [56, 128], one DVE multiply by the
  DMA-broadcast rstd producing the fp16 matmul operand, one small DMA for
  the [2, 1024] rank-2 rows.
- All matmul operands fp16 (1 col/cycle on PE); gate activations fp16 out;
  cell ops on DVE with fp16 inputs (2x DVE mode where operands allow);
  cell state c kept in fp32 for accuracy.
- ACT engine is the bottleneck (10 activations/step of [128,1024]); the
  two layers' emission is interleaved (all 8 gate sigmoids before the two
  tanh(c)) so ACT never stalls on the DVE cell chain.

PSUM: tag "pg" 3x[128,1024] f32 (6 banks) for gates/head, tag "px"
2x[128,128] (2 banks) for transposes/setup.
"""
import sys
sys.path.insert(0, '/opt/trn_rl_repo')

import numpy as np

B, T, F, H, D1, D2, OUT = 8192, 90, 7, 128, 128, 64, 30
NCORES = 8
BL = B // NCORES
G4 = 4 * H
NH = BL // 512
QB = BL // 128
EPS = 1e-5
LA = 3  # prepass lookahead (steps)


def _build(nc, T_steps=T, dbg=False):
    import concourse.tile as tile
    from concourse import mybir
    from concourse.masks import make_identity

    f32 = mybir.dt.float32
    f32r = mybir.dt.float32r
    fp16 = mybir.dt.float16
    AF = mybir.ActivationFunctionType
    ALU = mybir.AluOpType

    TS = T_steps

    # ---------------- DRAM I/O ----------------
    x_d = nc.dram_tensor("x", [BL, T, F], f32, kind="ExternalInput")
    W_in_d = nc.dram_tensor("W_in", [H, F], f32, kind="ExternalInput")
    b_in_d = nc.dram_tensor("b_in", [H], f32, kind="ExternalInput")
    g_in_d = nc.dram_tensor("g_in", [H], f32, kind="ExternalInput")
    be_in_d = nc.dram_tensor("be_in", [H], f32, kind="ExternalInput")
    Wih_d = [nc.dram_tensor("Wih0", [G4, H], f32, kind="ExternalInput"),
             nc.dram_tensor("Wih1", [G4, H], f32, kind="ExternalInput")]
    Whh_d = [nc.dram_tensor("Whh0", [G4, H], f32, kind="ExternalInput"),
             nc.dram_tensor("Whh1", [G4, H], f32, kind="ExternalInput")]
    bih_d = [nc.dram_tensor("bih0", [G4], f32, kind="ExternalInput"),
             nc.dram_tensor("bih1", [G4], f32, kind="ExternalInput")]
    bhh_d = [nc.dram_tensor("bhh0", [G4], f32, kind="ExternalInput"),
             nc.dram_tensor("bhh1", [G4], f32, kind="ExternalInput")]
    g_ln_d = nc.dram_tensor("g_ln", [H], f32, kind="ExternalInput")
    be_ln_d = nc.dram_tensor("be_ln", [H], f32, kind="ExternalInput")
    W_d1_d = nc.dram_tensor("W_d1", [D1, H], f32, kind="ExternalInput")
    b_d1_d = nc.dram_tensor("b_d1", [D1], f32, kind="ExternalInput")
    W_d2_d = nc.dram_tensor("W_d2", [D2, D1], f32, kind="ExternalInput")
    b_d2_d = nc.dram_tensor("b_d2", [D2], f32, kind="ExternalInput")
    W_d3_d = nc.dram_tensor("W_d3", [OUT, D2], f32, kind="ExternalInput")
    b_d3_d = nc.dram_tensor("b_d3", [OUT], f32, kind="ExternalInput")
    out_d = nc.dram_tensor("out", [BL, OUT], f32, kind="ExternalOutput")
    if dbg:
        dbg_xbr = nc.dram_tensor("dbg_xbr", [F, BL], f32, kind="ExternalOutput")
        dbg_rn = nc.dram_tensor("dbg_rn", [2, BL], f32, kind="ExternalOutput")
        dbg_h0 = nc.dram_tensor("dbg_h0", [H, BL], f32, kind="ExternalOutput")
        dbg_c0 = nc.dram_tensor("dbg_c0", [H, BL], f32, kind="ExternalOutput")
        dbg_g0 = nc.dram_tensor("dbg_g0", [H, BL], f32, kind="ExternalOutput")

    import contextlib
    with tile.TileContext(nc) as tc, contextlib.ExitStack() as ctx:
        singles = ctx.enter_context(tc.tile_pool(name="singles", bufs=1))
        trans = ctx.enter_context(tc.tile_pool(name="trans", bufs=2))
        small = ctx.enter_context(tc.tile_pool(name="small", bufs=1))
        feed = ctx.enter_context(tc.tile_pool(name="feed", bufs=LA + 1))
        ps_pg = ctx.enter_context(tc.tile_pool(name="ps_pg", bufs=3, space="PSUM"))
        ps_px = ctx.enter_context(tc.tile_pool(name="ps_px", bufs=2, space="PSUM"))
        dpool = ctx.enter_context(tc.tile_pool(name="dpool", bufs=1, space="DRAM"))

        def pg_tile(shape, name):
            return ps_pg.tile(shape, f32, tag="pg", name=name)

        def px_tile(shape, name):
            return ps_px.tile(shape, f32, tag="pxt", name=name)

        # ---------------- constants ----------------
        ident = singles.tile([128, 128], f32)
        make_identity(nc, ident)
        ones_row = singles.tile([1, 512], f32)
        nc.vector.memset(ones_row, 1.0)
        ones_col = singles.tile([128, 1], f32)
        nc.vector.memset(ones_col, 1.0)
        eps_col = singles.tile([T, 1], f32)
        nc.vector.memset(eps_col, EPS)

        def load_col(dram_vec, n, name):
            t_ = singles.tile([n, 1], f32, name=name, tag=name)
            nc.sync.dma_start(out=t_, in_=dram_vec[:].rearrange("(p o) -> p o", o=1))
            return t_

        # ---------------- x loads (issued first: everything gates on them) ----
        # xs[p, q, t, f] = x[128q+p, t, f]; contiguous (t,f) runs -> cheap DMA
        xs = singles.tile([128, QB, T, F], f32)
        nc.sync.dma_start(
            out=xs, in_=x_d[:, :, :].rearrange("(q p) t f -> p q t f", p=128))
        # fp16 copy: halves the per-step PE transpose cost (1.0 vs 2.0 c/row)
        xs16 = singles.tile([128, QB, T, F], fp16)
        nc.vector.tensor_copy(out=xs16, in_=xs)
        ident16 = singles.tile([128, 128], fp16)
        nc.vector.tensor_copy(out=ident16, in_=ident)

        g_in_c = load_col(g_in_d, H, "g_in_c")
        be_in_c = load_col(be_in_d, H, "be_in_c")
        b_in_c = load_col(b_in_d, H, "b_in_c")
        g_ln_c = load_col(g_ln_d, H, "g_ln_c")
        be_ln_c = load_col(be_ln_d, H, "be_ln_c")
        b_d1_c = load_col(b_d1_d, D1, "b_d1_c")
        b_d2_c = load_col(b_d2_d, D2, "b_d2_c")
        b_d3_c = load_col(b_d3_d, OUT, "b_d3_c")

        # bn1T [H, 2]: col0 = b_in, col1 = ones (for the rank-2 LN rows)
        bn1T = singles.tile([H, 2], f32)
        nc.vector.tensor_copy(out=bn1T[:, 0:1], in_=b_in_c)
        nc.vector.tensor_copy(out=bn1T[:, 1:2], in_=ones_col)

        w_in_raw = singles.tile([H, F], f32)
        nc.sync.dma_start(out=w_in_raw, in_=W_in_d[:, :])

        # ---------------- prepass: LN stats in [T, BL] layout ----------------
        # p' = W_in x + b_in per (h | b,t); over h:
        #   sum p'   = wsum . x + bsum
        #   sum p'^2 = x^T M x + 2 l^T x + c0,  M = W^T W, l = W^T b, c0=|b|^2
        p_m = px_tile([F, F], "pa")
        nc.tensor.matmul(p_m, w_in_raw, w_in_raw, start=True, stop=True)
        m_sb = small.tile([F, F], f32, tag="m_sb", name="m_sb")
        nc.vector.tensor_copy(out=m_sb, in_=p_m)
        p_ws = px_tile([1, F], "pa")
        nc.tensor.matmul(p_ws, ones_col, w_in_raw, start=True, stop=True)
        ws_sb = small.tile([1, F], f32, tag="ws_sb", name="ws_sb")
        nc.vector.tensor_copy(out=ws_sb, in_=p_ws)
        p_l = px_tile([1, F], "pa")
        nc.tensor.matmul(p_l, b_in_c, w_in_raw, start=True, stop=True)
        l_sb = small.tile([1, F], f32, tag="l_sb", name="l_sb")
        nc.vector.tensor_copy(out=l_sb, in_=p_l)
        p_sc = px_tile([1, 2], "pa")
        nc.tensor.matmul(p_sc[:, 0:1], b_in_c, b_in_c, start=True, stop=False,
                         skip_group_check=True)
        nc.tensor.matmul(p_sc[:, 1:2], ones_col, b_in_c, start=False, stop=True,
                         skip_group_check=True)
        sc_sb = small.tile([1, 2], f32, tag="sc_sb", name="sc_sb")
        nc.vector.tensor_copy(out=sc_sb, in_=p_sc)
        # stage stat constants to DRAM, then one partition-broadcast back
        NST = F * F + 2 * F + 2
        stat_dram = dpool.tile([1, NST], f32)
        nc.sync.dma_start(
            out=stat_dram[0:1, 0:F * F].rearrange("o (a b) -> (o a) b", a=F),
            in_=m_sb)
        nc.sync.dma_start(out=stat_dram[0:1, F * F:F * F + F], in_=ws_sb)
        nc.sync.dma_start(out=stat_dram[0:1, F * F + F:F * F + 2 * F], in_=l_sb)
        nc.sync.dma_start(out=stat_dram[0:1, F * F + 2 * F:NST], in_=sc_sb)
        statbc = singles.tile([T, NST], f32)
        nc.gpsimd.dma_start(out=statbc, in_=stat_dram[0:1, :].to_broadcast([T, NST]))
        mbc = statbc[:, 0:F * F]
        wbc = statbc[:, F * F:F * F + F]
        lbc = statbc[:, F * F + F:F * F + 2 * F]
        scbc = statbc[:, F * F + 2 * F:NST]

        # stats layout x16_tm[t, q, f, b] built from xs16 via PE transposes
        # (cheaper than the 92k-descriptor strided DMA reload of x)
        x16_tm = singles.tile([T, QB, F, 128], fp16)
        for q in range(QB):
            ptq = ps_px.tile([T, F, 128], fp16, tag="pxt", name="ptq")
            for fi in range(F):
                nc.tensor.transpose(ptq[:TS, fi, :], xs16[:, q, :TS, fi],
                                    ident16)
            nc.vector.tensor_copy(out=x16_tm[:TS, q, :, :], in_=ptq[:TS])

        # ---------------- weights: load + PE-transpose ----------------

        # transposed gate weights; layer-0 input weights only needed to
        # compose A7/B2 (f32 scratch), recurrent + layer-1 kept fp16.
        wihT0g = singles.tile([H, 4, H], f32)   # gamma-scaled Wih0^T
        wihT0f = singles.tile([H, 4, H], f32)   # unscaled (for beta fold)
        whhT = [singles.tile([H, 4, H], fp16, name="whhT0", tag="whhT0"),
                singles.tile([H, 4, H], fp16, name="whhT1", tag="whhT1")]
        wihT1 = singles.tile([H, 4, H], fp16)
        wih_raw = [trans.tile([H, 4, H], f32, tag="wraw", name=f"wih_raw{L}")
                   for L in range(2)]
        whh_raw = [trans.tile([H, 4, H], f32, tag="hraw", name=f"whh_raw{L}")
                   for L in range(2)]
        for L in range(2):
            nc.sync.dma_start(
                out=wih_raw[L],
                in_=Wih_d[L][:, :].rearrange("(c p) h -> p c h", p=H))
            nc.sync.dma_start(
                out=whh_raw[L],
                in_=Whh_d[L][:, :].rearrange("(c p) h -> p c h", p=H))
        for L in range(2):
            for cc in range(4):
                pt_w = px_tile([H, H], "tr_ps_w")
                nc.tensor.transpose(pt_w, wih_raw[L][:, cc, :], ident)
                if L == 0:
                    nc.scalar.copy(out=wihT0f[:, cc, :], in_=pt_w)
                    nc.scalar.mul(out=wihT0g[:, cc, :], in_=pt_w, mul=g_in_c)
                else:
                    nc.scalar.copy(out=wihT1[:, cc, :], in_=pt_w)
                pt_h = px_tile([H, H], "tr_ps_w")
                nc.tensor.transpose(pt_h, whh_raw[L][:, cc, :], ident)
                nc.scalar.copy(out=whhT[L][:, cc, :], in_=pt_h)

        # composed layer-0 input weights (fp16): A7 = Wih0g @ W_in,
        # B2 = Wih0g @ [b_in | 1] (separate tiles: compute engines may only
        # address partition ranges starting at 0/32/64)
        A7 = singles.tile([F, 4, H], fp16)
        B2 = singles.tile([2, 4, H], fp16)
        for cc in range(4):
            pa = px_tile([F, H], "pa")
            nc.tensor.matmul(pa, w_in_raw, wihT0g[:, cc, :], start=True, stop=True)
            nc.scalar.copy(out=A7[:, cc, :], in_=pa)
            pb2 = px_tile([2, H], "pa")
            nc.tensor.matmul(pb2, bn1T, wihT0g[:, cc, :], start=True, stop=True)
            nc.scalar.copy(out=B2[:, cc, :], in_=pb2)

        # gate biases beff[L] [128, 4]; layer-0 gets the beta fold Wih0 @ be_in
        beff = []
        for L in range(2):
            bt_ = singles.tile([H, 4], f32, name=f"beff{L}", tag=f"beff{L}")
            bih_sb = small.tile([H, 4], f32, tag="bload", name="bih_sb")
            nc.sync.dma_start(out=bih_sb,
                              in_=bih_d[L][:].rearrange("(c p) -> p c", p=H))
            bhh_sb = small.tile([H, 4], f32, tag="bload2", name="bhh_sb")
            nc.sync.dma_start(out=bhh_sb,
                              in_=bhh_d[L][:].rearrange("(c p) -> p c", p=H))
            nc.vector.tensor_add(out=bt_, in0=bih_sb, in1=bhh_sb)
            beff.append(bt_)
        for cc in range(4):
            pb = px_tile([H, 1], "pa")
            nc.tensor.matmul(pb, wihT0f[:, cc, :], be_in_c, start=True, stop=True)
            nc.vector.tensor_add(out=beff[0][:, cc:cc + 1],
                                 in0=beff[0][:, cc:cc + 1], in1=pb)

        # head weights
        def transpose_to(dst, src_ap, p, fdim):
            pt = px_tile([fdim, p], "pa")
            nc.tensor.transpose(pt, src_ap, ident[:p, :p])
            nc.scalar.copy(out=dst, in_=pt)

        wd1T = singles.tile([H, D1], fp16)
        wd1_raw = trans.tile([D1, H], f32, tag="wraw", name="wd1_raw")
        nc.sync.dma_start(out=wd1_raw, in_=W_d1_d[:, :])
        transpose_to(wd1T, wd1_raw, D1, H)
        wd2T = singles.tile([D1, D2], fp16)
        wd2_raw = trans.tile([D2, D1], f32, tag="hraw", name="wd2_raw")
        nc.sync.dma_start(out=wd2_raw, in_=W_d2_d[:, :])
        transpose_to(wd2T, wd2_raw, D2, D1)
        wd3T = singles.tile([D2, OUT], fp16)
        wd3_raw = trans.tile([OUT, D2], f32, tag="wraw", name="wd3_raw")
        nc.sync.dma_start(out=wd3_raw, in_=W_d3_d[:, :])
        transpose_to(wd3T, wd3_raw, OUT, D2)
        ones16 = singles.tile([128, 1], fp16)
        nc.vector.tensor_copy(out=ones16, in_=ones_col)

        def xf(fi):
            return x16_tm[:TS, :, fi, :]

        def g3(ap):
            return ap.rearrange("t (q b) -> t q b", b=128)

        # fp16 products + pairwise quadratic form (ts 4x / TT 2x DVE modes;
        # the 3-operand scalar_tensor_tensor has no fast mode so avoid it)
        nmu_all = singles.tile([T, BL], f32)
        r_all = singles.tile([T, BL], f32)

        def dot_chain(dst, coef_col):
            # dst = sum_f coef[f] * x_f  via 7 scaled products + 6 adds
            pa_ = trans.tile([T, BL], fp16, tag="tg", name="st_pa")
            pb_ = trans.tile([T, BL], fp16, tag="sig_o", name="st_pb")
            nc.vector.tensor_scalar_mul(out=g3(pa_[:TS]), in0=xf(0),
                                        scalar1=coef_col(0))
            for fi in range(1, F):
                nc.vector.tensor_scalar_mul(out=g3(pb_[:TS]), in0=xf(fi),
                                            scalar1=coef_col(fi))
                nc.vector.tensor_add(out=dst[:TS] if fi == F - 1 else pa_[:TS],
                                     in0=pa_[:TS], in1=pb_[:TS])

        acc = trans.tile([T, BL], fp16, tag="sig_i", name="st_acc")
        dot_chain(acc, lambda fi: wbc[:TS, fi:fi + 1])
        # nmu = -(acc + bsum)/H
        nc.vector.tensor_scalar(out=nmu_all[:TS], in0=acc[:TS],
                                scalar1=scbc[:TS, 1:2], scalar2=-1.0 / H,
                                op0=ALU.add, op1=ALU.mult)
        lin = trans.tile([T, BL], fp16, tag="u", name="st_lin")
        dot_chain(lin, lambda fi: lbc[:TS, fi:fi + 1])
        # qform = sum_{i<=j} c_ij x_i x_j, c_ij = M_ii or 2 M_ij; fold the
        # lin term in as a virtual pair via the accumulate chain
        qacc = trans.tile([T, BL], fp16, tag="sig_f", name="st_qacc")
        ts_ = trans.tile([T, BL], fp16, tag="tg", name="st_ts")
        tp_ = trans.tile([T, BL], fp16, tag="sig_o", name="st_tp")
        first = True
        for fi in range(F):
            for fj in range(fi, F):
                mcol = mbc[:TS, fi * F + fj:fi * F + fj + 1]
                if fi == fj:
                    nc.vector.tensor_scalar_mul(out=g3(ts_[:TS]), in0=xf(fj),
                                                scalar1=mcol)
                else:
                    nc.vector.tensor_scalar(out=g3(ts_[:TS]), in0=xf(fj),
                                            scalar1=mcol, scalar2=2.0,
                                            op0=ALU.mult, op1=ALU.mult)
                nc.vector.tensor_tensor(
                    out=g3(tp_[:TS]) if not first else g3(qacc[:TS]),
                    in0=g3(ts_[:TS]), in1=xf(fi), op=ALU.mult)
                if not first:
                    nc.vector.tensor_add(out=qacc[:TS], in0=qacc[:TS],
                                         in1=tp_[:TS])
                first = False
        # qacc += 2 * lin
        nc.vector.tensor_scalar_mul(out=lin[:TS], in0=lin[:TS], scalar1=2.0)
        nc.vector.tensor_add(out=qacc[:TS], in0=qacc[:TS], in1=lin[:TS])
        # var = (q + c0)/H - mu^2 ; r = 1/sqrt(var+eps)  (f32 tail)
        var32 = trans.tile([T, BL], f32, tag="v_", name="var32")
        nc.vector.tensor_scalar(out=var32[:TS], in0=qacc[:TS],
                                scalar1=scbc[:TS, 0:1], scalar2=1.0 / H,
                                op0=ALU.add, op1=ALU.mult)
        musq = trans.tile([T, BL], f32, tag="v32", name="musq")
        nc.vector.tensor_tensor(out=musq[:TS], in0=nmu_all[:TS],
                                in1=nmu_all[:TS], op=ALU.mult)
        nc.vector.tensor_sub(out=var32[:TS], in0=var32[:TS], in1=musq[:TS])
        nc.scalar.activation(out=r_all[:TS], in_=var32[:TS], func=AF.Sqrt,
                             bias=eps_col[:TS], scale=1.0)
        nc.vector.reciprocal(out=r_all[:TS], in_=r_all[:TS])
        # fp16 rows for the prepass broadcast + rank-2 matmul rows
        r16 = trans.tile([T, BL], fp16, tag="tg", name="r16")
        nc.vector.tensor_copy(out=r16[:TS], in_=r_all[:TS])
        nmr16 = trans.tile([T, BL], fp16, tag="sig_o", name="nmr16")
        nc.vector.tensor_tensor(out=nmr16[:TS], in0=nmu_all[:TS],
                                in1=r_all[:TS], op=ALU.mult)
        # DRAM staging: fp16 [r; -mu*r] rows
        rnm16_dram = dpool.tile([2, T, BL], fp16)
        nc.sync.dma_start(out=rnm16_dram[0, :TS], in_=r16[:TS])
        nc.sync.dma_start(out=rnm16_dram[1, :TS], in_=nmr16[:TS])

        # ---------------- states ----------------
        h1 = singles.tile([H, BL], fp16, name="h1", tag="h1")
        c = [singles.tile([H, BL], f32, name="c0", tag="c0"),
             singles.tile([H, BL], f32, name="c1", tag="c1")]
        zinit = trans.tile([H, BL], f32, tag="u", name="zinit")
        nc.vector.memset(zinit, 0.0)
        h0_prev = trans.tile([H, BL], fp16, tag="h0", name="h0_init")
        nc.vector.tensor_copy(out=h0_prev, in_=zinit)
        nc.vector.tensor_copy(out=h1, in_=zinit)
        for L in range(2):
            nc.vector.memset(c[L], 0.0)

        # ---------------- per-step prepass ----------------
        def prepass(t):
            # feature-major x_t: 8 PE transposes into [7, 1024] column blocks
            px = ps_px.tile([F, BL], fp16, tag="pxt", name="pxt_main")
            for q in range(QB):
                nc.tensor.transpose(px[:, q * 128:(q + 1) * 128],
                                    xs16[:, q, t, :], ident16)
            # rstd broadcast over the 7 feature rows (fp16 -> 2x DVE mode)
            rb = feed.tile([F, BL], fp16, tag="rb", name="rb", bufs=2)
            nc.gpsimd.dma_start(
                out=rb, in_=rnm16_dram[0, t:t + 1, :].to_broadcast([F, BL]))
            xbr = feed.tile([F, BL], fp16, tag="xbr", name="xbr")
            nc.vector.tensor_tensor(out=xbr, in0=px, in1=rb, op=ALU.mult)
            # rank-2 rows [r; -mu*r]
            rn = feed.tile([2, BL], fp16, tag="rn", name="rn")
            nc.gpsimd.dma_start(out=rn, in_=rnm16_dram[:, t, :])
            return xbr, rn

        feed_bufs = {}
        for t in range(min(LA, TS)):
            feed_bufs[t] = prepass(t)

        # ---------------- main loop ----------------
        sig_funcs = [AF.Sigmoid, AF.Sigmoid, AF.Tanh, AF.Sigmoid]

        def gate_l0(gc, xbr, rn, hprev):
            pg = pg_tile([H, BL], "pg_gates")
            for hc in range(NH):
                sl = slice(hc * 512, (hc + 1) * 512)
                nc.tensor.matmul(pg[:, sl], A7[:, gc, :], xbr[:, sl],
                                 start=True, stop=False,
                                 skip_group_check=True)
                nc.tensor.matmul(pg[:, sl], B2[:, gc, :], rn[:, sl],
                                 start=False, stop=False,
                                 skip_group_check=True)
                nc.tensor.matmul(pg[:, sl], whhT[0][:, gc, :], hprev[:, sl],
                                 start=False, stop=(hc == NH - 1),
                                 skip_group_check=True)
            g_out = trans.tile([H, BL], fp16, tag=f"g{gc}", name=f"g0_{gc}")
            nc.scalar.activation(out=g_out, in_=pg, func=sig_funcs[gc],
                                 bias=beff[0][:, gc:gc + 1], scale=1.0)
            return g_out

        def gate_l1_mm(gc, hprev):
            pg = pg_tile([H, BL], "pg_gates")
            for hc in range(NH):
                sl = slice(hc * 512, (hc + 1) * 512)
                nc.tensor.matmul(pg[:, sl], wihT1[:, gc, :], hprev[:, sl],
                                 start=True, stop=False,
                                 skip_group_check=True)
                nc.tensor.matmul(pg[:, sl], whhT[1][:, gc, :], h1[:, sl],
                                 start=False, stop=True,
                                 skip_group_check=True)
            return pg

        def gate_l1_act(gc, pg):
            g_out = trans.tile([H, BL], fp16, tag=f"g{gc}", name=f"g1_{gc}")
            nc.scalar.activation(out=g_out, in_=pg, func=sig_funcs[gc],
                                 bias=beff[1][:, gc:gc + 1], scale=1.0)
            return g_out

        def iteration(t, h0_prev):
            """One fused iteration: layer-0 step t plus layer-1 step t-1
            (g1 state carried in nonlocal l1g).  Emission order keeps the
            ACT stream dense: sigma0 x4, sigma1 i/f, tanh_c0, sigma1 g/o,
            tanh_c1."""
            xbr_t, rn_t = feed_bufs.pop(t)
            # L0 gates (PE + ACT)
            g0 = [gate_l0(gc, xbr_t, rn_t, h0_prev) for gc in range(4)]
            # L1 matmuls for step t-1 + first two activations
            pg1 = [gate_l1_mm(gc, h0_prev) for gc in range(4)] if t > 0 else None
            g1 = [gate_l1_act(gc, pg1[gc]) for gc in range(2)] if t > 0 else None
            # L0 cell chain: u, v, c
            u0 = trans.tile([H, BL], fp16, tag="u16", name="u0")
            nc.vector.tensor_tensor(out=u0, in0=g0[0], in1=g0[2], op=ALU.mult)
            v0 = trans.tile([H, BL], f32, tag="v32", name="v0")
            nc.vector.tensor_tensor(out=v0, in0=g0[1], in1=c[0], op=ALU.mult)
            nc.vector.tensor_add(out=c[0], in0=u0, in1=v0)
            tc0 = trans.tile([H, BL], fp16, tag="tc16", name="tc0")
            nc.scalar.activation(out=tc0, in_=c[0], func=AF.Tanh, scale=1.0)
            h0_new = trans.tile([H, BL], fp16, tag="h0", name="h0_new")
            nc.vector.tensor_tensor(out=h0_new, in0=g0[3], in1=tc0, op=ALU.mult)
            # prepass for t+LA behind the gate matmuls
            if t + LA < TS:
                feed_bufs[t + LA] = prepass(t + LA)
            if t > 0:
                g1 = g1 + [gate_l1_act(gc, pg1[gc]) for gc in range(2, 4)]
                u1 = trans.tile([H, BL], fp16, tag="u16", name="u1")
                nc.vector.tensor_tensor(out=u1, in0=g1[0], in1=g1[2],
                                        op=ALU.mult)
                v1 = trans.tile([H, BL], f32, tag="v32", name="v1")
                nc.vector.tensor_tensor(out=v1, in0=g1[1], in1=c[1],
                                        op=ALU.mult)
                nc.vector.tensor_add(out=c[1], in0=u1, in1=v1)
                tc1 = trans.tile([H, BL], fp16, tag="tc16", name="tc1")
                nc.scalar.activation(out=tc1, in_=c[1], func=AF.Tanh, scale=1.0)
                nc.vector.tensor_tensor(out=h1, in0=g1[3], in1=tc1,
                                        op=ALU.mult)
            if dbg and t == 0:
                dxbr = trans.tile([F, BL], f32, tag="u16", name="dxbr")
                nc.vector.tensor_copy(out=dxbr, in_=xbr_t)
                nc.sync.dma_start(out=dbg_xbr[:, :], in_=dxbr)
                drn = trans.tile([2, BL], f32, tag="v32", name="drn")
                nc.vector.tensor_copy(out=drn, in_=rn_t)
                nc.sync.dma_start(out=dbg_rn[:, :], in_=drn)
                dh0 = trans.tile([H, BL], f32, tag="v32", name="dh0")
                nc.vector.tensor_copy(out=dh0, in_=h0_new)
                nc.sync.dma_start(out=dbg_h0[:, :], in_=dh0)
                nc.sync.dma_start(out=dbg_c0[:, :], in_=c[0])
                dg0 = trans.tile([H, BL], f32, tag="u16", name="dg0")
                nc.vector.tensor_copy(out=dg0, in_=g0[0])
                nc.sync.dma_start(out=dbg_g0[:, :], in_=dg0)
            return h0_new

        for t in range(TS):
            h0_prev = iteration(t, h0_prev)
        # final layer-1 step (t = TS-1)
        pg1 = [gate_l1_mm(gc, h0_prev) for gc in range(4)]
        g1 = [gate_l1_act(gc, pg1[gc]) for gc in range(4)]
        u1 = trans.tile([H, BL], fp16, tag="u16", name="u1f")
        nc.vector.tensor_tensor(out=u1, in0=g1[0], in1=g1[2], op=ALU.mult)
        v1 = trans.tile([H, BL], f32, tag="v32", name="v1f")
        nc.vector.tensor_tensor(out=v1, in0=g1[1], in1=c[1], op=ALU.mult)
        nc.vector.tensor_add(out=c[1], in0=u1, in1=v1)
        tc1 = trans.tile([H, BL], fp16, tag="tc16", name="tc1f")
        nc.scalar.activation(out=tc1, in_=c[1], func=AF.Tanh, scale=1.0)
        nc.vector.tensor_tensor(out=h1, in0=g1[3], in1=tc1, op=ALU.mult)

        # ---------------- head ----------------
        # LN sums per q-block -> [8, 128] combine ops (cheap free size)
        sqh = trans.tile([H, BL], fp16, tag="g0", name="sqh")
        nc.vector.tensor_tensor(out=sqh, in0=h1, in1=h1, op=ALU.mult)
        ps_s1 = pg_tile([1, BL], "ps_s1")
        ps_s2 = pg_tile([1, BL], "ps_s2")
        for hc in range(NH):
            sl = slice(hc * 512, (hc + 1) * 512)
            nc.tensor.matmul(ps_s1[:, sl], ones16, h1[:, sl],
                             start=True, stop=True, skip_group_check=True)
            nc.tensor.matmul(ps_s2[:, sl], ones16, sqh[:, sl],
                             start=True, stop=True, skip_group_check=True)
        nmu_h = small.tile([1, BL], fp16, tag="nmu_h", name="nmu_h")
        nc.vector.tensor_scalar_mul(out=nmu_h, in0=ps_s1, scalar1=-1.0 / H)
        musq_h = small.tile([1, BL], fp16, tag="musq", name="musq_h")
        nc.vector.tensor_tensor(out=musq_h, in0=nmu_h, in1=nmu_h, op=ALU.mult)
        v_h = small.tile([1, BL], f32, tag="v_h", name="v_h")
        nc.vector.tensor_scalar_mul(out=v_h, in0=ps_s2, scalar1=1.0 / H)
        nc.vector.tensor_sub(out=v_h, in0=v_h, in1=musq_h)
        nc.scalar.activation(out=v_h, in_=v_h, func=AF.Sqrt,
                             bias=eps_col[0:1], scale=1.0)
        nc.vector.reciprocal(out=v_h, in_=v_h)
        hstat_dram = dpool.tile([1, BL], fp16)
        hstat32_dram = dpool.tile([1, BL], f32)
        nc.sync.dma_start(out=hstat_dram[0:1, :], in_=nmu_h)
        nc.sync.dma_start(out=hstat32_dram[0:1, :], in_=v_h)
        nmbc = trans.tile([H, BL], fp16, tag="g1", name="nmbc")
        nc.gpsimd.dma_start(
            out=nmbc, in_=hstat_dram[0:1, :].to_broadcast([H, BL]))
        rhbc = trans.tile([H, BL], f32, tag="g2", name="rhbc")
        nc.sync.dma_start(
            out=rhbc, in_=hstat32_dram[0:1, :].to_broadcast([H, BL]))
        t1 = trans.tile([H, BL], fp16, tag="g3", name="t1")
        nc.vector.tensor_tensor(out=t1, in0=h1, in1=nmbc, op=ALU.add)
        t2 = trans.tile([H, BL], fp16, tag="g0", name="t2")
        nc.vector.tensor_tensor(out=t2, in0=t1, in1=rhbc, op=ALU.mult)
        last = trans.tile([H, BL], fp16, tag="g1", name="last")
        nc.vector.tensor_scalar(out=last, in0=t2, scalar1=g_ln_c,
                                scalar2=be_ln_c, op0=ALU.mult, op1=ALU.add)
        pd1 = pg_tile([D1, BL], "pd1")
        for hc in range(NH):
            sl = slice(hc * 512, (hc + 1) * 512)
            nc.tensor.matmul(pd1[:, sl], wd1T, last[:, sl],
                             start=True, stop=True, skip_group_check=True)
        d1 = trans.tile([D1, BL], fp16, tag="g2", name="d1")
        nc.scalar.activation(out=d1, in_=pd1, func=AF.Relu, bias=b_d1_c, scale=1.0)
        pd2 = pg_tile([D2, BL], "pd2")
        for hc in range(NH):
            sl = slice(hc * 512, (hc + 1) * 512)
            nc.tensor.matmul(pd2[:, sl], wd2T, d1[:, sl],
                             start=True, stop=True, skip_group_check=True)
        d2 = trans.tile([D2, BL], fp16, tag="g3", name="d2")
        nc.scalar.activation(out=d2, in_=pd2, func=AF.Relu, bias=b_d2_c, scale=1.0)
        pd3 = pg_tile([OUT, BL], "pd3")
        for hc in range(NH):
            sl = slice(hc * 512, (hc + 1) * 512)
            nc.tensor.matmul(pd3[:, sl], wd3T, d2[:, sl],
                             start=True, stop=True, skip_group_check=True)
        o3 = trans.tile([OUT, BL], f32, tag="sig_f", name="o3")
        nc.scalar.activation(out=o3, in_=pd3, func=AF.Identity, bias=b_d3_c,
                             scale=1.0)
        outT = singles.tile([128, QB, OUT], f32)
        for q in range(QB):
            pot = px_tile([128, OUT], "pot")
            nc.tensor.transpose(pot, o3[:, q * 128:(q + 1) * 128],
                                ident[:OUT, :OUT])
            nc.vector.tensor_copy(out=outT[:, q, :], in_=pot)
        nc.sync.dma_start(
            out=out_d[:, :].rearrange("(q p) c -> p q c", p=128),
            in_=outT)
    return nc


_CACHE = {}


def _get_runner():
    if "runner" in _CACHE:
        return _CACHE["runner"]
    import jax
    from jax.sharding import Mesh, PartitionSpec
    from jax.experimental.shard_map import shard_map
    import concourse.bacc as bacc
    import concourse.mybir as mybir
    from concourse.bass2jax import install_neuronx_cc_hook, _bass_exec_p, \
        partition_id_tensor

    nc = bacc.Bacc()
    _build(nc)
    nc.compile()
    install_neuronx_cc_hook()

    partition_name = nc.partition_id_tensor.name if nc.partition_id_tensor else None
    in_names, out_names, out_avals, zero_outs = [], [], [], []
    for alloc in nc.m.functions[0].allocations:
        if not isinstance(alloc, mybir.MemoryLocationSet):
            continue
        name = alloc.memorylocations[0].name
        if alloc.kind == "ExternalInput":
            if name != partition_name:
                in_names.append(name)
        elif alloc.kind == "ExternalOutput":
            out_names.append(name)
            shape = tuple(alloc.tensor_shape)
            dtype = mybir.dt.np(alloc.dtype)
            out_avals.append(jax.core.ShapedArray(shape, dtype))
            zero_outs.append(np.zeros(shape, dtype))
    n_params = len(in_names)
    all_in_names = in_names + out_names + ([partition_name] if partition_name else [])

    def _body(*args):
        operands = list(args)
        if partition_name is not None:
            operands.append(partition_id_tensor())
        outs = _bass_exec_p.bind(
            *operands,
            out_avals=tuple(out_avals),
            in_names=tuple(all_in_names),
            out_names=tuple(out_names),
            lowering_input_output_aliases=(),
            sim_require_finite=False,
            sim_require_nnan=False,
            nc=nc,
        )
        return tuple(outs)

    devices = jax.devices()[:NCORES]
    mesh = Mesh(np.asarray(devices), ("core",))
    in_specs = (PartitionSpec("core"),) * (n_params + len(out_names))
    out_specs = (PartitionSpec("core"),) * len(out_names)
    sharded = jax.jit(
        shard_map(_body, mesh=mesh, in_specs=in_specs, out_specs=out_specs,
                  check_rep=False),
        keep_unused=True)
    _CACHE["runner"] = (sharded, in_names, out_names, zero_outs)
    return _CACHE["runner"]


def kernel(**inputs) -> np.ndarray:
    sharded, in_names, out_names, zero_outs = _get_runner()
    inp = {k: np.ascontiguousarray(np.asarray(v), dtype=np.float32)
           for k, v in inputs.items()}

    def core_val(name, ci):
        if name == "x":
            return inp["x"][ci * BL:(ci + 1) * BL]
        return inp[name]

    concat_in = [
        np.concatenate([core_val(n, ci) for ci in range(NCORES)], axis=0)
        for n in in_names
    ]
    concat_zeros = [
        np.zeros((NCORES * z.shape[0], *z.shape[1:]), z.dtype) for z in zero_outs
    ]
    import jax
    out_arrs = sharded(*concat_in, *concat_zeros)
    jax.block_until_ready(out_arrs)
    oi = out_names.index("out")
    full = np.asarray(out_arrs[oi]).reshape(B, OUT)
    return full.astype(np.float32)
